# revision 22
# baseline (speedup 1.0000x reference)
"""Bass/Trainium2 kernel for nn_CriterionSA (CAM/gridPAM CKA loss).

Self-contained: hardcodes shapes/sharding for the
B=16, C=256, H=W=80 problem on 8 NeuronCores.

Sharding (v2 — "chunk-sharded", minimal host->device traffic):
  The raw features are shipped exactly once (~26MB/core):
    - xc:   core j owns grid chunks 3j..3j+2 in natural [C,N] layout for
            ALL 16 samples, both tensors (25.2MB).
    - x24o: chunk 24 for the core's own 2 samples (1MB) — PAM chunk-24 is
            sample-split as before.
    - x24s: chunk 24, positions 32j..32j+32, ALL samples (1MB) — so the
            union of per-core CAM spatial slices covers all 6400 positions.
  All repartitioning happens on device:
    - X^T tiles are produced by TensorE transposes (identity matmul); the
      gamma*bv-biased transposed copy is cached to DRAM for the PAM
      residual path.
    - CAM energy: each core accumulates per-sample [C,C] energy partials
      over its own spatial positions, then one 8MB ReduceScatter(add)
      hands each core the full energies of its 2 own samples.
    - CAM attention rows are AllGathered (1MB/core) as before; each core
      then computes the CAM output restricted to its 800 spatial
      positions for all 16 samples and a partial [128,128] gram.
  Outputs are small: per-chunk/per-core [128,128] gram partials (host
  extracts/sums the 16x16 diagonal blocks) + chunk-24 PAM features.
"""

import os
import sys

import numpy as np

_REPO = "/opt/trn_rl_repo"
if _REPO not in sys.path:
    sys.path.insert(0, _REPO)

import concourse.bacc as bacc
import concourse.mybir as mybir
import concourse.tile as tile
from concourse import bass_utils

F32 = mybir.dt.float32
EXP = mybir.ActivationFunctionType.Exp
IDN = mybir.ActivationFunctionType.Identity
AX = mybir.AxisListType.X
ADD = mybir.AluOpType.add

NCORES = 8
B, C, H, W = 16, 256, 80, 80
CK = 32          # C // 8
NCH = 256        # spatial positions per grid chunk (16x16)
TAU = 1.0

IN_SPECS = {
    "xc":   (3, 2, 16, 2, 128, 256),  # (ci, t, b, cb, c_low, n)
    "x24o": (2, 2, 2, 128, 256),      # (t, own-b, cb, c_low, n)
    "x24s": (2, 16, 2, 128, 32),      # (t, b, cb, c_low, ns)
    "wqT":  (2, 128, 32),
    "wkT":  (2, 128, 32),
    "wvT":  (2, 128, 256),            # (gamma_pam * Wv)^T
    "bq4":  (128, 1),
    "i128": (128, 128),
    "gicam": (128, 128),              # gamma_cam * I
    "gbv2": (1, 256),                 # gamma_pam*bv row
    "one1": (1, 128),                 # ones row (rank-1 bias matmul)
}
OUT_SPECS = {
    "gpam": (3, 2, 128, 128),         # per (ci, t) chunk gram supers
    "gcam": (2, 128, 128),            # per t CAM gram partial
    "c24r": (2, 2, 2, 128, 256),      # (t, own-b, m-tile, m_low, c) PAM R^T
}


# --------------------------------------------------------------------------
# device program
# --------------------------------------------------------------------------

def _emit_softmax_attn_T(nc, sb, ep, eye_ap, n_i, tag):
    """From energy tile ep [128, 512] (two 256-wide row-blocks along free),
    produce (expE sbuf [128,512], dg sbuf [128,256]) where dg holds two 128x128
    diagonal blocks diag(eye_scale / Z). Softmax rows are the PARTITION dim of
    each 256-block; normalization uses exp(E - rowmax)."""
    nm = sb.tile([128, 2], F32, tag=f"nm{tag}", name=f"nm{tag}")
    nc.vector.tensor_reduce(
        nm[:], ep.rearrange("p (i j) -> p i j", i=2), AX,
        op=mybir.AluOpType.max, negate=True)
    expe = sb.tile([128, 512], F32, tag=f"expe{tag}", name=f"expe{tag}")
    zz = sb.tile([128, 2], F32, tag=f"zz{tag}", name=f"zz{tag}")
    for i in range(n_i):
        nc.scalar.activation(
            expe[:, i * 256:(i + 1) * 256], ep[:, i * 256:(i + 1) * 256],
            EXP, bias=nm[:, i:i + 1], scale=1.0, accum_out=zz[:, i:i + 1])
    rr = sb.tile([128, 2], F32, tag=f"rr{tag}", name=f"rr{tag}")
    nc.vector.reciprocal(rr[:, 0:n_i], zz[:, 0:n_i])
    dg = sb.tile([128, 256], F32, tag=f"dg{tag}", name=f"dg{tag}")
    for i in range(n_i):
        nc.vector.tensor_scalar_mul(
            dg[:, i * 128:(i + 1) * 128], eye_ap, rr[:, i:i + 1])
    return expe, dg


def _emit_pam_sample(nc, cst, sbs, psa, pso, q_sl, k_sl, xf, boff,
                     ep2_pool, row_base=0):
    """One PAM attention sample. q_sl/k_sl: [32,256] APs (same base partition,
    = row_base). xf: 2 natural c-tiles; boff: free offset of this sample in xf.
    The residual X^T + gamma*bv is generated on the fly: TensorE transpose of
    the xf blocks plus a rank-1 ones x gbv matmul.
    Returns op_ PSUM tile [128, 512] = R^T, layout (m-tile 2)(c 256)."""
    ep2 = ep2_pool.tile([128, 512], F32, tag="ep2", name="ep2")
    for ib in range(2):
        nc.tensor.matmul(
            ep2[:, ib * 256:(ib + 1) * 256],
            lhsT=q_sl[:, ib * 128:(ib + 1) * 128], rhs=k_sl,
            start=True, stop=True, tile_position=(row_base, 0))
    expe, dg = _emit_softmax_attn_T(nc, sbs, ep2, cst["i128"][:], 2, "p")
    avp = psa.tile([128, 1024], F32, tag="avp", name="avp")
    # A^T (normalized) blocks: avp[:, jb*256+ib*128] = expE[ib-rows, jb-cols]^T * diag
    for jb in range(2):
        for ib in range(2):
            nc.tensor.matmul(
                avp[:, jb * 256 + ib * 128: jb * 256 + ib * 128 + 128],
                lhsT=expe[:, ib * 256 + jb * 128: ib * 256 + jb * 128 + 128],
                rhs=dg[:, ib * 128:(ib + 1) * 128], start=True, stop=True)
    # v^T = Xf^T @ (gamma Wv)^T
    for jb in range(2):
        for cb in range(2):
            nc.tensor.matmul(
                avp[:, 512 + jb * 256: 512 + (jb + 1) * 256],
                lhsT=xf[cb][:, boff + jb * 128: boff + jb * 128 + 128],
                rhs=cst["wvT"][cb][:], start=(cb == 0), stop=(cb == 1))
    av = sbs.tile([128, 1024], F32, tag="av", name="av")
    nc.scalar.copy(av[:, 0:512], avp[:, 0:512])
    nc.vector.tensor_copy(av[:, 512:1024], avp[:, 512:1024])
    op_ = pso.tile([128, 512], F32, tag="opam", name="opam")
    for mb in range(2):
        for jb in range(2):
            nc.tensor.matmul(
                op_[:, mb * 256:(mb + 1) * 256],
                lhsT=av[:, jb * 256 + mb * 128: jb * 256 + mb * 128 + 128],
                rhs=av[:, 512 + jb * 256: 512 + (jb + 1) * 256],
                start=(jb == 0), stop=False)
        # residual: += X^T (transpose of xf m-block) ...
        for cb in range(2):
            nc.tensor.matmul(
                op_[:, mb * 256 + cb * 128: mb * 256 + cb * 128 + 128],
                lhsT=xf[cb][:, boff + mb * 128: boff + mb * 128 + 128],
                rhs=cst["i128"][:], start=False, stop=False)
        # ... += gamma*bv (rank-1 broadcast over the m partition rows)
        nc.tensor.matmul(
            op_[:, mb * 256:(mb + 1) * 256],
            lhsT=cst["one1"][:], rhs=cst["gbv2"][0:1, 0:256],
            start=False, stop=True)
    return op_


def _emit_qk(nc, cst, psq, xf, qtb, ktb):
    """q/k passes over a 16-sample chunk unit (samples col-packed 4-wide)."""
    for which, wt, dst in (("q", "wqT", qtb), ("k", "wkT", ktb)):
        qp = psq.tile([128, 1024], F32, tag="qkp", name="qkp")
        for w in range(8):
            r_ = 32 * (w % 4)
            fo = (w // 4) * 512
            for kb in range(2):
                nc.tensor.matmul(
                    qp[r_:r_ + 32, fo:fo + 512],
                    lhsT=cst[wt][kb][:],
                    rhs=xf[kb][:, w * 512:(w + 1) * 512],
                    start=(kb == 0), stop=(kb == 1),
                    tile_position=(0, r_))
        if which == "q":
            nc.scalar.activation(dst[:], qp[:], IDN,
                                 bias=cst["bq4"][:], scale=1.0)
        else:
            nc.scalar.copy(dst[:], qp[:])


def _emit_program(nc, I, O):
    phases = os.environ.get("CRIT_PHASES", "abc")
    with tile.TileContext(nc) as tc:
        cpool = tc.alloc_tile_pool(name="const", bufs=1)
        dram = tc.alloc_tile_pool(name="ccdram", bufs=1, space="DRAM")
        cst = {}
        for nm_ in ("wqT", "wkT", "wvT"):
            cst[nm_] = []
            for kb in range(2):
                t = cpool.tile(list(IN_SPECS[nm_][1:]), F32, name=f"{nm_}{kb}")
                nc.sync.dma_start(t[:], I[nm_][kb])
                cst[nm_].append(t)
        for nm_ in ("bq4", "i128", "gicam", "gbv2", "one1"):
            t = cpool.tile(list(IN_SPECS[nm_]), F32, name=nm_)
            nc.sync.dma_start(t[:], I[nm_][:])
            cst[nm_] = t

        # (sh, t, bo, p, cb, d) — partition-major rows so SBUF<->DRAM DMAs
        # need no partition transposes
        rs_in = dram.tile([8, 2, 2, 128, 512], F32, name="rs_in")
        rs_out = dram.tile([2, 2, 128, 512], F32, name="rs_out")
        atnb = dram.tile([8, 128, 256], F32, name="atnb")
        atng = dram.tile([8, 8, 128, 256], F32, name="atng", addr_space="Shared")

        for _rep in range(int(os.environ.get("CRIT_REPS", "1"))):
            _emit_body(tc, nc, I, O, cst, rs_in, rs_out, atnb, atng, phases)

        cpool.release()
        dram.release()


def _emit_body(tc, nc, I, O, cst, rs_in, rs_out, atnb, atng, phases):
    if True:
        # ---------------- Phase A: transposes + energy partials ----------
        if "a" in phases:
            with tc.tile_pool(name="pacc", bufs=1) as pacc, \
                    tc.tile_pool(name="pa", bufs=3) as pa, \
                    tc.tile_pool(name="paT", bufs=2, space="PSUM") as psT, \
                    tc.tile_pool(name="paE", bufs=2, space="PSUM") as psE:
                eacc = pacc.tile([128, 16384], F32, name="eacc")  # (t,b)*(cb,d)
                for ci in range(3):
                    for t in range(2):
                        for b in range(16):
                            xn = pa.tile([128, 512], F32, tag="xn", name="xn")
                            for cb in range(2):
                                nc.sync.dma_start(
                                    xn[:, cb * 256:(cb + 1) * 256],
                                    I["xc"][ci, t, b, cb])
                            tp = psT.tile([128, 512], F32, tag="tp", name="tp")
                            for nt in range(2):
                                for cb in range(2):
                                    nc.tensor.matmul(
                                        tp[:, nt * 256 + cb * 128:
                                           nt * 256 + cb * 128 + 128],
                                        lhsT=xn[:, cb * 256 + nt * 128:
                                                cb * 256 + nt * 128 + 128],
                                        rhs=cst["i128"][:],
                                        start=True, stop=True)
                            xtr = pa.tile([128, 512], F32, tag="xtr", name="xtr")
                            nc.scalar.copy(xtr[:], tp[:])
                            ep = psE.tile([128, 512], F32, tag="ep", name="ep")
                            for cb in range(2):
                                for nt in range(2):
                                    nc.tensor.matmul(
                                        ep[:, cb * 256:(cb + 1) * 256],
                                        lhsT=xtr[:, nt * 256 + cb * 128:
                                                 nt * 256 + cb * 128 + 128],
                                        rhs=xtr[:, nt * 256:(nt + 1) * 256],
                                        start=(nt == 0), stop=(nt == 1))
                            off = (t * 16 + b) * 512
                            if ci == 0:
                                nc.vector.tensor_copy(
                                    eacc[:, off:off + 512], ep[:])
                            else:
                                nc.vector.tensor_tensor(
                                    eacc[:, off:off + 512],
                                    eacc[:, off:off + 512], ep[:], op=ADD)
                # chunk-24 position slice: all samples, 32 positions
                for t in range(2):
                    for b in range(16):
                        xs = pa.tile([128, 64], F32, tag="xs", name="xs")
                        for cb in range(2):
                            nc.sync.dma_start(
                                xs[:, cb * 32:(cb + 1) * 32],
                                I["x24s"][t, b, cb])
                        tps = psT.tile([32, 256], F32, tag="tps", name="tps")
                        for cb in range(2):
                            nc.tensor.matmul(
                                tps[:, cb * 128:(cb + 1) * 128],
                                lhsT=xs[:, cb * 32:(cb + 1) * 32],
                                rhs=cst["i128"][:], start=True, stop=True)
                        xsr = pa.tile([32, 256], F32, tag="xsr", name="xsr")
                        nc.scalar.copy(xsr[:], tps[:])
                        eps = psE.tile([128, 512], F32, tag="ep", name="ep")
                        for cb in range(2):
                            nc.tensor.matmul(
                                eps[:, cb * 256:(cb + 1) * 256],
                                lhsT=xsr[:, cb * 128:(cb + 1) * 128],
                                rhs=xsr[:], start=True, stop=True)
                        off = (t * 16 + b) * 512
                        nc.vector.tensor_tensor(
                            eacc[:, off:off + 512],
                            eacc[:, off:off + 512], eps[:], op=ADD)
                # stage eacc -> rs_in ordered by owner shard
                for sh in range(8):
                    for t in range(2):
                        for bo in range(2):
                            src = eacc[:, (t * 16 + 2 * sh + bo) * 512:
                                       (t * 16 + 2 * sh + bo + 1) * 512]
                            nc.sync.dma_start(rs_in[sh, t, bo], src)

        if "a" in phases:
            nc.gpsimd.collective_compute(
                "ReduceScatter", ADD,
                replica_groups=[list(range(NCORES))],
                ins=[rs_in.opt()], outs=[rs_out.opt()])

        # ---------------- Phase B: PAM chunks (units 0..2 first) ----------
        with tc.tile_pool(name="pxf", bufs=2) as sbx, \
                tc.tile_pool(name="pX", bufs=1) as sbX, \
                tc.tile_pool(name="pqk", bufs=1) as sbqk, \
                tc.tile_pool(name="psmall", bufs=2) as sbs, \
                tc.tile_pool(name="qkps", bufs=1, space="PSUM") as psq, \
                tc.tile_pool(name="eps", bufs=1, space="PSUM") as pse, \
                tc.tile_pool(name="avps", bufs=1, space="PSUM") as psa, \
                tc.tile_pool(name="ops", bufs=2, space="PSUM") as pso, \
                tc.tile_pool(name="gps", bufs=1, space="PSUM") as psg:

            def pam_unit(u):
                ci, t = u // 2, u % 2
                xf = []
                for cb in range(2):
                    xft = sbx.tile([128, 4096], F32, tag=f"xf{cb}",
                                   name=f"xf{cb}")
                    nc.sync.dma_start(
                        xft[:],
                        I["xc"][ci, t].rearrange(
                            "b cb p n -> cb p b n")[cb])
                    xf.append(xft)
                qtb = sbqk.tile([128, 1024], F32, tag="qtb", name="qtb")
                ktb = sbqk.tile([128, 1024], F32, tag="ktb", name="ktb")
                _emit_qk(nc, cst, psq, xf, qtb, ktb)
                X = sbX.tile([128, 8192], F32, tag="X", name="X")
                for b in range(16):
                    w = b // 2
                    rb = 32 * (w % 4)
                    fo = (w // 4) * 512 + (b % 2) * 256
                    op_ = _emit_pam_sample(
                        nc, cst, sbs, psa, pso,
                        qtb[rb:rb + 32, fo:fo + 256], ktb[rb:rb + 32, fo:fo + 256],
                        xf, b * 256, pse, row_base=rb)
                    nc.vector.tensor_copy(
                        X.rearrange("p (mt d b2) -> p mt d b2", mt=2, b2=16)[:, :, :, b],
                        op_.rearrange("p (mt d) -> p mt d", mt=2))
                gp = psg.tile([128, 128], F32, tag="gp", name="gp")
                for s in range(64):
                    nc.tensor.matmul(
                        gp[:], lhsT=X[:, s * 128:(s + 1) * 128],
                        rhs=X[:, s * 128:(s + 1) * 128],
                        start=(s == 0), stop=(s == 63))
                gps = sbs.tile([128, 128], F32, tag="gpsb", name="gpsb")
                nc.scalar.copy(gps[:], gp[:])
                nc.sync.dma_start(O["gpam"][ci, t], gps[:])

            for u in range(3 if "b" in phases else 0):
                pam_unit(u)

            # -------- softmax of own CAM energies + attn AllGather --------
            if "a" in phases:
                eo = sbs.tile([128, 2048], F32, tag="eo", name="eo")
                for t in range(2):
                    for bo in range(2):
                        nc.sync.dma_start(
                            eo[:, (t * 2 + bo) * 512:(t * 2 + bo + 1) * 512],
                            rs_out[t, bo])
                for t in range(2):
                    for bo in range(2):
                        ecur = eo[:, (t * 2 + bo) * 512:(t * 2 + bo + 1) * 512]
                        # softmax of (min - E) rows == softmax(max_d E - E)
                        mn = sbs.tile([128, 2], F32, tag="mnc", name="mnc")
                        for cb in range(2):
                            nc.vector.tensor_reduce(
                                mn[:, cb:cb + 1],
                                ecur[:, cb * 256:(cb + 1) * 256], AX,
                                op=mybir.AluOpType.min)
                        expe = sbs.tile([128, 512], F32, tag="expec",
                                        name="expec")
                        zz = sbs.tile([128, 2], F32, tag="zzc", name="zzc")
                        for cb in range(2):
                            nc.scalar.activation(
                                expe[:, cb * 256:(cb + 1) * 256],
                                ecur[:, cb * 256:(cb + 1) * 256],
                                EXP, bias=mn[:, cb:cb + 1], scale=-1.0,
                                accum_out=zz[:, cb:cb + 1])
                        rr = sbs.tile([128, 2], F32, tag="rrc", name="rrc")
                        nc.vector.reciprocal(rr[:], zz[:])
                        dgc = sbs.tile([128, 256], F32, tag="dgc", name="dgc")
                        for cb in range(2):
                            nc.vector.tensor_scalar_mul(
                                dgc[:, cb * 128:(cb + 1) * 128],
                                cst["gicam"][:], rr[:, cb:cb + 1])
                        # reuse the PAM ep2 PSUM buffer (phase-B pool budget)
                        atc = pse.tile([128, 512], F32, tag="ep2", name="ep2")
                        for dt in range(2):
                            for cb in range(2):
                                nc.tensor.matmul(
                                    atc[:, dt * 256 + cb * 128:
                                        dt * 256 + cb * 128 + 128],
                                    lhsT=expe[:, cb * 256 + dt * 128:
                                              cb * 256 + dt * 128 + 128],
                                    rhs=dgc[:, cb * 128:(cb + 1) * 128],
                                    start=True, stop=True)
                        atcs = sbs.tile([128, 512], F32, tag="atcs",
                                        name="atcs")
                        nc.vector.tensor_copy(atcs[:], atc[:])
                        for dt in range(2):
                            nc.sync.dma_start(
                                atnb[bo * 4 + t * 2 + dt],
                                atcs[:, dt * 256:(dt + 1) * 256])
                nc.gpsimd.collective_compute(
                    "AllGather", mybir.AluOpType.bypass,
                    replica_groups=[list(range(NCORES))],
                    ins=[atnb.opt()], outs=[atng.opt()])

            for u in range(3 if "b" in phases else 0, 6 if "b" in phases else 0):
                pam_unit(u)

            # chunk 24: 2 own samples, R^T straight to DRAM
            for t in range(2 if "b" in phases else 0):
                for bo in range(2):
                    xf4 = []
                    for cb in range(2):
                        x4 = sbs.tile([128, 256], F32, tag=f"xf4{cb}",
                                      name=f"xf4{cb}")
                        nc.sync.dma_start(x4[:], I["x24o"][t, bo, cb])
                        xf4.append(x4)
                    qtb4 = sbs.tile([32, 256], F32, tag="qtb4", name="qtb4")
                    ktb4 = sbs.tile([32, 256], F32, tag="ktb4", name="ktb4")
                    for which, wt, dst in (("q", "wqT", qtb4), ("k", "wkT", ktb4)):
                        qp4 = psq.tile([128, 1024], F32, tag="qkp", name="qkp")
                        for kb in range(2):
                            nc.tensor.matmul(
                                qp4[0:32, 0:256], lhsT=cst[wt][kb][:],
                                rhs=xf4[kb][:], start=(kb == 0), stop=(kb == 1))
                        if which == "q":
                            nc.scalar.activation(dst[:], qp4[0:32, 0:256], IDN,
                                                 bias=cst["bq4"][0:32, :],
                                                 scale=1.0)
                        else:
                            nc.scalar.copy(dst[:], qp4[0:32, 0:256])
                    op4 = _emit_pam_sample(
                        nc, cst, sbs, psa, pso, qtb4[:], ktb4[:],
                        xf4, 0, pse)
                    op4s = sbs.tile([128, 512], F32, tag="op4s", name="op4s")
                    nc.vector.tensor_copy(op4s[:], op4[:])
                    for mt in range(2):
                        nc.sync.dma_start(
                            O["c24r"][t, bo, mt],
                            op4s[:, mt * 256:(mt + 1) * 256])

    # ---------------- Phase C: CAM out-slice + partial grams ----------
    with tc.tile_pool(name="c2at", bufs=1) as sb2a, \
            tc.tile_pool(name="c2x", bufs=1) as sb2x, \
            tc.tile_pool(name="c2n", bufs=3) as sb2n, \
            tc.tile_pool(name="c2ops", bufs=2, space="PSUM") as ps2o, \
            tc.tile_pool(name="c2gps", bufs=1, space="PSUM") as ps2g:
        for t in range(2 if "c" in phases else 0):
            atn = sb2a.tile([128, 8192], F32, tag="atn", name="atn")
            for b in range(16):
                for dt in range(2):
                    nc.sync.dma_start(
                        atn[:, (b * 2 + dt) * 256:(b * 2 + dt + 1) * 256],
                        atng[b // 2, (b % 2) * 4 + t * 2 + dt])
            gacc = sb2n.tile([128, 128], F32, tag="gacc", name="gacc")
            for ci in range(3):
                Xw = sb2x.tile([128, 8192], F32, tag="Xw", name="Xw")
                for b in range(16):
                    xn = sb2n.tile([128, 512], F32, tag="xn2", name="xn2")
                    for cb in range(2):
                        nc.sync.dma_start(
                            xn[:, cb * 256:(cb + 1) * 256],
                            I["xc"][ci, t, b, cb])
                    ocp = ps2o.tile([128, 1024], F32, tag="ocp", name="ocp")
                    for cb in range(2):
                        for dt in range(2):
                            nc.tensor.matmul(
                                ocp[:, cb * 512:cb * 512 + 256],
                                lhsT=atn[:, (b * 2 + dt) * 256 + cb * 128:
                                         (b * 2 + dt) * 256 + cb * 128 + 128],
                                rhs=xn[:, dt * 256:(dt + 1) * 256],
                                start=(dt == 0), stop=False)
                        nc.tensor.matmul(
                            ocp[:, cb * 512:cb * 512 + 256],
                            lhsT=cst["i128"][:],
                            rhs=xn[:, cb * 256:(cb + 1) * 256],
                            start=False, stop=True)
                    dst = Xw.rearrange(
                        "p (cb n b2) -> p cb n b2", cb=2, b2=16)[:, :, :, b]
                    src = ocp.rearrange("p (cb n) -> p cb n", cb=2)[:, :, 0:256]
                    if b % 2 == 0:
                        nc.vector.tensor_copy(dst, src)
                    else:
                        nc.scalar.copy(dst, src)
                gcp = ps2g.tile([128, 128], F32, tag="gcp", name="gcp")
                for s in range(64):
                    nc.tensor.matmul(
                        gcp[:], lhsT=Xw[:, s * 128:(s + 1) * 128],
                        rhs=Xw[:, s * 128:(s + 1) * 128],
                        start=(s == 0), stop=(s == 63))
                if ci == 0:
                    nc.vector.tensor_copy(gacc[:], gcp[:])
                else:
                    nc.vector.tensor_tensor(gacc[:], gacc[:], gcp[:], op=ADD)
            # chunk-24 position-slice window
            Xs = sb2x.tile([128, 1024], F32, tag="Xs", name="Xs")
            for b in range(16):
                xs = sb2n.tile([128, 64], F32, tag="xs2", name="xs2")
                for cb in range(2):
                    nc.sync.dma_start(
                        xs[:, cb * 32:(cb + 1) * 32], I["x24s"][t, b, cb])
                oc2 = ps2o.tile([128, 64], F32, tag="oc2", name="oc2")
                for cb in range(2):
                    for dt in range(2):
                        nc.tensor.matmul(
                            oc2[:, cb * 32:(cb + 1) * 32],
                            lhsT=atn[:, (b * 2 + dt) * 256 + cb * 128:
                                     (b * 2 + dt) * 256 + cb * 128 + 128],
                            rhs=xs[:, dt * 32:(dt + 1) * 32],
                            start=(dt == 0), stop=False)
                    nc.tensor.matmul(
                        oc2[:, cb * 32:(cb + 1) * 32],
                        lhsT=cst["i128"][:],
                        rhs=xs[:, cb * 32:(cb + 1) * 32],
                        start=False, stop=True)
                dst = Xs.rearrange(
                    "p (cb ns b2) -> p cb ns b2", cb=2, b2=16)[:, :, :, b]
                src = oc2.rearrange("p (cb ns) -> p cb ns", cb=2)
                if b % 2 == 0:
                    nc.vector.tensor_copy(dst, src)
                else:
                    nc.scalar.copy(dst, src)
            gcp = ps2g.tile([128, 128], F32, tag="gcp", name="gcp")
            for s in range(8):
                nc.tensor.matmul(
                    gcp[:], lhsT=Xs[:, s * 128:(s + 1) * 128],
                    rhs=Xs[:, s * 128:(s + 1) * 128],
                    start=(s == 0), stop=(s == 7))
            nc.vector.tensor_tensor(gacc[:], gacc[:], gcp[:], op=ADD)
            gcs = sb2n.tile([128, 128], F32, tag="gcs", name="gcs")
            nc.scalar.copy(gcs[:], gacc[:])
            nc.sync.dma_start(O["gcam"][t], gcs[:])


_PROG = None


def _get_prog():
    global _PROG
    if _PROG is None:
        nc = bacc.Bacc("TRN2", target_bir_lowering=False, debug=False,
                       num_devices=NCORES)
        I = {n: nc.dram_tensor(n, list(s), F32, kind="ExternalInput").ap()
             for n, s in IN_SPECS.items()}
        O = {n: nc.dram_tensor(n, list(s), F32, kind="ExternalOutput").ap()
             for n, s in OUT_SPECS.items()}
        _emit_program(nc, I, O)
        nc.compile()
        _PROG = nc
    return _PROG


# --------------------------------------------------------------------------
# host side
# --------------------------------------------------------------------------

def _make_in_maps(feat_S, feat_T, Wq, bq, Wk, bk, Wv, bv, gammacam, gammapam):
    gp = float(np.asarray(gammapam).reshape(-1)[0])
    gc = float(np.asarray(gammacam).reshape(-1)[0])
    gbv = (gp * np.asarray(bv, np.float32)).astype(np.float32)

    # chunk-major global rearrange: [25, 2, 16, 2, 128, 256]
    A = np.empty((25, 2, 16, 256, 256), np.float32)
    for t, X in enumerate((feat_S, feat_T)):
        A[:, t] = (np.asarray(X, np.float32)
                   .reshape(B, C, 5, 16, 5, 16)
                   .transpose(2, 4, 0, 1, 3, 5)
                   .reshape(25, B, C, 256))
    A = A.reshape(25, 2, 16, 2, 128, 256)

    consts = {
        "wqT": np.ascontiguousarray(np.asarray(Wq, np.float32).T.reshape(2, 128, CK)),
        "wkT": np.ascontiguousarray(np.asarray(Wk, np.float32).T.reshape(2, 128, CK)),
        "wvT": np.ascontiguousarray(
            (gp * np.asarray(Wv, np.float32)).T.reshape(2, 128, C)),
        "bq4": np.ascontiguousarray(np.tile(np.asarray(bq, np.float32), 4)[:, None]),
        "i128": np.eye(128, dtype=np.float32),
        "gicam": (gc * np.eye(128)).astype(np.float32),
        "gbv2": np.ascontiguousarray(gbv[None, :]),
        "one1": np.ones((1, 128), np.float32),
    }

    in_maps = []
    for j in range(NCORES):
        m = dict(consts)
        m["xc"] = A[3 * j:3 * j + 3]
        m["x24o"] = np.ascontiguousarray(A[24][:, 2 * j:2 * j + 2])
        m["x24s"] = np.ascontiguousarray(A[24][:, :, :, :, 32 * j:32 * j + 32])
        in_maps.append(m)
    return in_maps


def _diag16(gfull):
    """gfull: [..., 128, 128] partials; f64-sum partials then diagonal blocks."""
    gf = gfull.astype(np.float64).reshape(-1, 128, 128).sum(axis=0)
    g = np.zeros((16, 16), np.float64)
    for r in range(8):
        g += gf[16 * r:16 * r + 16, 16 * r:16 * r + 16]
    return g


def _cka_loss(KS, KT):
    def cgram(K):
        rm = K.mean(axis=1, keepdims=True)
        cm = K.mean(axis=0, keepdims=True)
        return K - rm - cm + K.mean()
    cX, cY = cgram(KS), cgram(KT)
    hsic = float((cX * cY).sum())
    v1 = float(np.sqrt((cX * cX).sum()))
    v2 = float(np.sqrt((cY * cY).sum()))
    return -np.log(np.abs(hsic / (v1 * v2)) + 1e-8)


def _postprocess(results):
    losses = []
    for c in range(24):
        j, ci = divmod(c, 3)
        res = results[j]
        KS = _diag16(res["gpam"][ci, 0])
        KT = _diag16(res["gpam"][ci, 1])
        losses.append(_cka_loss(KS, KT))
    # chunk 24 on host
    FS = np.empty((B, 2 * 128 * 256), np.float32)
    FT = np.empty((B, 2 * 128 * 256), np.float32)
    for j in range(NCORES):
        for bo in range(2):
            FS[2 * j + bo] = results[j]["c24r"][0, bo].reshape(-1)
            FT[2 * j + bo] = results[j]["c24r"][1, bo].reshape(-1)
    FS = FS.astype(np.float64)
    FT = FT.astype(np.float64)
    KS24 = FS @ FS.T
    KT24 = FT @ FT.T
    losses.append(_cka_loss(KS24, KT24))
    loss_PAM = float(np.mean(losses))

    KSc = np.zeros((16, 16), np.float64)
    KTc = np.zeros((16, 16), np.float64)
    for j in range(NCORES):
        KSc += _diag16(results[j]["gcam"][0])
        KTc += _diag16(results[j]["gcam"][1])
    loss_CAM = float(_cka_loss(KSc, KTc))
    return np.float32(loss_CAM), np.float32(loss_PAM)


def _run_sim(nc, in_maps):
    from concourse.bass_interp import MultiCoreSim
    sim = MultiCoreSim(nc, num_cores=NCORES)
    cores = list(sim.cores.values())
    for j, core in enumerate(cores):
        for name, arr in in_maps[j].items():
            core.tensor(name)[:] = arr
    sim.simulate()
    return [{n: core.tensor(n).copy() for n in OUT_SPECS} for core in cores]


_LAST_EXEC_NS = None


def kernel(**inputs):
    global _LAST_EXEC_NS
    nc = _get_prog()
    in_maps = _make_in_maps(**{k: np.asarray(v) for k, v in inputs.items()})
    if os.environ.get("CRIT_BACKEND", "hw") == "sim":
        results = _run_sim(nc, in_maps)
    else:
        res = bass_utils.run_bass_kernel_spmd(
            nc, in_maps, core_ids=list(range(NCORES)),
            trace=os.environ.get("CRIT_TRACE", "0") == "1")
        results = res.results
        _LAST_EXEC_NS = res.exec_time_ns
    return _postprocess(results)


# revision 36
# speedup vs baseline: 1.2185x; 1.2185x over previous
"""Bass/Trainium2 kernel for nn_CriterionSA (CAM/gridPAM CKA loss).

Self-contained: hardcodes shapes/sharding for the
B=16, C=256, H=W=80 problem on 8 NeuronCores.

Sharding (v3 — chunk-sharded bf16 shipping, ~13MB/core host->device):
  Raw features are shipped once, in bf16, grid-chunk partitioned:
    - xc:   core j owns grid chunks 3j..3j+2 in natural [C,N] layout for
            ALL 16 samples, both tensors.
    - x24o: chunk 24 for the core's own 2 samples (PAM chunk-24 is
            sample-split).
    - x24s: chunk 24, positions 32j..32j+32, ALL samples (CAM spatial
            coverage).
  On-device repartitioning:
    - X^T tiles come from DMA crossbar transposes (bf16); per-sample [C,C]
      CAM energy partials accumulate in PSUM (f32) over the core's
      positions, staged per-sample into an 8MB ReduceScatter(add) that
      hands each core the full energies of its 2 own samples.
    - CAM attention rows (f32) are AllGathered (1MB/core); each core then
      computes the CAM output over its 800 spatial positions for all 16
      samples (f32) and a partial [128,128] gram.
  PAM runs fully in bf16 (inputs/q/k/v/attention/supers) with f32 PSUM;
  the gamma*bv bias is folded into the v^T copy (attention rows sum to 1).
  CAM energy/attention/output stay f32 (softmax exponent sensitivity).
"""

import os
import sys

import numpy as np

_REPO = "/opt/trn_rl_repo"
if _REPO not in sys.path:
    sys.path.insert(0, _REPO)

import ml_dtypes
import concourse.bacc as bacc
import concourse.mybir as mybir
import concourse.tile as tile
from concourse import bass_utils

F32 = mybir.dt.float32
BF16 = mybir.dt.bfloat16
EXP = mybir.ActivationFunctionType.Exp
IDN = mybir.ActivationFunctionType.Identity
AX = mybir.AxisListType.X
ADD = mybir.AluOpType.add

NCORES = 8
B, C, H, W = 16, 256, 80, 80
CK = 32          # C // 8
TAU = 1.0

IN_SPECS = {
    # feature shards: f32 for the CAM paths (the CAM CKA loss sits at
    # correlation-distance ~5e-5 from 1 and percent-shifts under bf16
    # feature rounding); PAM-only data ships bf16.
    "xc":   ((3, 2, 16, 2, 128, 256), F32),   # (ci, t, b, cb, c_low, n)
    "x24o": ((2, 2, 2, 128, 256), BF16),      # (t, own-b, cb, c_low, n)
    "x24s": ((2, 16, 2, 128, 32), F32),       # (t, b, cb, c_low, ns)
    # weights / constants
    "wqT":  ((2, 128, 32), BF16),
    "wkT":  ((2, 128, 32), BF16),
    "wvT":  ((2, 128, 256), BF16),            # (gamma_pam * Wv)^T
    "bq4":  ((128, 1), F32),
    "i128": ((128, 128), F32),
    "i128b": ((128, 128), BF16),
    "gicam": ((128, 128), F32),               # gamma_cam * I
    "gbv512": ((128, 512), F32),              # gamma_pam*bv bcast (2x 256)
}
OUT_SPECS = {
    "gpam": (3, 2, 128, 128),         # per (ci, t) chunk gram supers
    "gcam": (2, 128, 128),            # per t CAM gram partial
    "c24r": (2, 2, 2, 128, 256),      # (t, own-b, m-tile, m_low, c) PAM R^T
}


# --------------------------------------------------------------------------
# device program
# --------------------------------------------------------------------------

def _emit_softmax_attn_T(nc, sb, ep, eye_ap, n_i, tag):
    """From energy PSUM tile ep [128, 512] (two 256-wide row-blocks along
    free), produce (expE f32 [128,512], dg f32 [128,256]) where dg holds two
    128x128 diagonal blocks diag(1/Z). Softmax rows are the PARTITION dim of
    each 256-block; normalization uses exp(E - rowmax). Kept in f32 so the
    normalized attention is only rounded to bf16 once (at the av copy)."""
    nm = sb.tile([128, 2], F32, tag=f"nm{tag}", name=f"nm{tag}")
    nc.vector.tensor_reduce(
        nm[:], ep.rearrange("p (i j) -> p i j", i=2), AX,
        op=mybir.AluOpType.max, negate=True)
    expe = sb.tile([128, 512], F32, tag=f"expe{tag}", name=f"expe{tag}")
    zz = sb.tile([128, 2], F32, tag=f"zz{tag}", name=f"zz{tag}")
    for i in range(n_i):
        nc.scalar.activation(
            expe[:, i * 256:(i + 1) * 256], ep[:, i * 256:(i + 1) * 256],
            EXP, bias=nm[:, i:i + 1], scale=1.0, accum_out=zz[:, i:i + 1])
    rr = sb.tile([128, 2], F32, tag=f"rr{tag}", name=f"rr{tag}")
    nc.vector.reciprocal(rr[:, 0:n_i], zz[:, 0:n_i])
    dg = sb.tile([128, 256], F32, tag=f"dg{tag}", name=f"dg{tag}")
    for i in range(n_i):
        nc.vector.tensor_scalar_mul(
            dg[:, i * 128:(i + 1) * 128], eye_ap, rr[:, i:i + 1])
    return expe, dg


def _emit_pam_sample(nc, cst, sbs, psa, pso, q_sl, k_sl, xf, boff,
                     ep2_pool, row_base=0):
    """One PAM attention sample (bf16 pipeline, f32 PSUM). q_sl/k_sl:
    [32,256] bf16 APs at base partition row_base. xf: 2 natural bf16 c-tiles;
    boff: free offset of this sample in xf. The residual X^T comes from
    TensorE transposes of the xf blocks; gamma*bv is folded into the v^T
    copy (attention rows sum to 1).
    Returns op_ PSUM tile [128, 512] = R^T, layout (m-tile 2)(c 256)."""
    ep2 = ep2_pool.tile([128, 512], F32, tag="ep2", name="ep2")
    for ib in range(2):
        nc.tensor.matmul(
            ep2[:, ib * 256:(ib + 1) * 256],
            lhsT=q_sl[:, ib * 128:(ib + 1) * 128], rhs=k_sl,
            start=True, stop=True, tile_position=(row_base, 0))
    expe, dg = _emit_softmax_attn_T(nc, sbs, ep2, cst["i128"][:], 2, "p")
    avp = psa.tile([128, 1024], F32, tag="avp", name="avp")
    # A^T (normalized) blocks: avp[:, jb*256+ib*128] = expE[ib-rows, jb-cols]^T * diag
    for jb in range(2):
        for ib in range(2):
            nc.tensor.matmul(
                avp[:, jb * 256 + ib * 128: jb * 256 + ib * 128 + 128],
                lhsT=expe[:, ib * 256 + jb * 128: ib * 256 + jb * 128 + 128],
                rhs=dg[:, ib * 128:(ib + 1) * 128], start=True, stop=True)
    # v^T = Xf^T @ (gamma Wv)^T
    for jb in range(2):
        for cb in range(2):
            nc.tensor.matmul(
                avp[:, 512 + jb * 256: 512 + (jb + 1) * 256],
                lhsT=xf[cb][:, boff + jb * 128: boff + jb * 128 + 128],
                rhs=cst["wvT"][cb][:], start=(cb == 0), stop=(cb == 1))
    av = sbs.tile([128, 1024], BF16, tag="av", name="av")
    nc.scalar.copy(av[:, 0:512], avp[:, 0:512])
    # v^T + gamma*bv (rows of attention sum to 1, so the bias folds here)
    nc.vector.tensor_tensor(
        av[:, 512:1024], avp[:, 512:1024], cst["gbv512"][:], op=ADD)
    op_ = pso.tile([128, 512], F32, tag="opam", name="opam")
    for mb in range(2):
        for jb in range(2):
            nc.tensor.matmul(
                op_[:, mb * 256:(mb + 1) * 256],
                lhsT=av[:, jb * 256 + mb * 128: jb * 256 + mb * 128 + 128],
                rhs=av[:, 512 + jb * 256: 512 + (jb + 1) * 256],
                start=(jb == 0), stop=False)
        # residual: += X^T (transpose of xf m-block)
        for cb in range(2):
            nc.tensor.matmul(
                op_[:, mb * 256 + cb * 128: mb * 256 + cb * 128 + 128],
                lhsT=xf[cb][:, boff + mb * 128: boff + mb * 128 + 128],
                rhs=cst["i128b"][:], start=False, stop=(cb == 1))
    return op_


def _emit_qk(nc, cst, psq, xf, qtb, ktb):
    """q/k passes over a 16-sample chunk unit (samples col-packed 4-wide)."""
    for which, wt, dst in (("q", "wqT", qtb), ("k", "wkT", ktb)):
        qp = psq.tile([128, 1024], F32, tag="qkp", name="qkp")
        for w in range(8):
            r_ = 32 * (w % 4)
            fo = (w // 4) * 512
            for kb in range(2):
                nc.tensor.matmul(
                    qp[r_:r_ + 32, fo:fo + 512],
                    lhsT=cst[wt][kb][:],
                    rhs=xf[kb][:, w * 512:(w + 1) * 512],
                    start=(kb == 0), stop=(kb == 1),
                    tile_position=(0, r_))
        if which == "q":
            nc.scalar.activation(dst[:], qp[:], IDN,
                                 bias=cst["bq4"][:], scale=1.0)
        else:
            nc.scalar.copy(dst[:], qp[:])


def _emit_program(nc, I, O):
    phases = os.environ.get("CRIT_PHASES", "abc")
    with tile.TileContext(nc) as tc:
        cpool = tc.alloc_tile_pool(name="const", bufs=1)
        dram = tc.alloc_tile_pool(name="ccdram", bufs=1, space="DRAM")
        cst = {}
        for nm_ in ("wqT", "wkT", "wvT"):
            cst[nm_] = []
            for kb in range(2):
                t = cpool.tile(list(IN_SPECS[nm_][0][1:]), IN_SPECS[nm_][1],
                               name=f"{nm_}{kb}")
                nc.sync.dma_start(t[:], I[nm_][kb])
                cst[nm_].append(t)
        for nm_ in ("bq4", "i128", "i128b", "gicam", "gbv512"):
            t = cpool.tile(list(IN_SPECS[nm_][0]), IN_SPECS[nm_][1], name=nm_)
            nc.sync.dma_start(t[:], I[nm_][:])
            cst[nm_] = t

        # (sh, t, bo, p, (cb d)) — partition-major rows, f32 energies
        rs_in = dram.tile([8, 2, 2, 128, 512], F32, name="rs_in")
        rs_out = dram.tile([2, 2, 128, 512], F32, name="rs_out")
        atnb = dram.tile([8, 128, 256], F32, name="atnb")
        atng = dram.tile([8, 8, 128, 256], F32, name="atng", addr_space="Shared")

        for _rep in range(int(os.environ.get("CRIT_REPS", "1"))):
            _emit_body(tc, nc, I, O, cst, rs_in, rs_out, atnb, atng, phases)

        cpool.release()
        dram.release()


def _emit_body(tc, nc, I, O, cst, rs_in, rs_out, atnb, atng, phases):
    # ---------------- Phase A: XBAR transposes + energy partials ----------
    if "a" in phases:
        with tc.tile_pool(name="pa", bufs=3) as pa, \
                tc.tile_pool(name="paT", bufs=2, space="PSUM") as psT, \
                tc.tile_pool(name="paE", bufs=2, space="PSUM") as psE:
            for t in range(2):
                for b in range(16):
                    xtrs = []
                    for ci in range(3):
                        xn = pa.tile([128, 512], F32, tag=f"xn{ci}",
                                     name=f"xn{ci}")
                        for cb in range(2):
                            nc.sync.dma_start(
                                xn[:, cb * 256:(cb + 1) * 256],
                                I["xc"][ci, t, b, cb])
                        tp = psT.tile([128, 512], F32, tag="tp", name="tp")
                        for nt in range(2):
                            for cb in range(2):
                                nc.tensor.matmul(
                                    tp[:, nt * 256 + cb * 128:
                                       nt * 256 + cb * 128 + 128],
                                    lhsT=xn[:, cb * 256 + nt * 128:
                                            cb * 256 + nt * 128 + 128],
                                    rhs=cst["i128"][:], start=True, stop=True)
                        xtr = pa.tile([128, 512], F32, tag=f"xtr{ci}",
                                      name=f"xtr{ci}")
                        nc.scalar.copy(xtr[:], tp[:])
                        xtrs.append(xtr)
                    # chunk-24 position slice (TensorE transpose, 32 pos)
                    xs = pa.tile([128, 64], F32, tag="xs", name="xs")
                    for cb in range(2):
                        nc.sync.dma_start(
                            xs[:, cb * 32:(cb + 1) * 32], I["x24s"][t, b, cb])
                    tps = psT.tile([32, 256], F32, tag="tps", name="tps")
                    for cb in range(2):
                        nc.tensor.matmul(
                            tps[:, cb * 128:(cb + 1) * 128],
                            lhsT=xs[:, cb * 32:(cb + 1) * 32],
                            rhs=cst["i128"][:], start=True, stop=True)
                    xsr = pa.tile([32, 256], F32, tag="xsr", name="xsr")
                    nc.vector.tensor_copy(xsr[:], tps[:])
                    # energy accumulation: one window's chain at a time
                    # (PSUM start zeroes the whole bank)
                    et = psE.tile([128, 512], F32, tag="et", name="et")
                    for cb in range(2):
                        for ci in range(3):
                            for nt in range(2):
                                nc.tensor.matmul(
                                    et[:, cb * 256:(cb + 1) * 256],
                                    lhsT=xtrs[ci][:, nt * 256 + cb * 128:
                                                  nt * 256 + cb * 128 + 128],
                                    rhs=xtrs[ci][:, nt * 256:(nt + 1) * 256],
                                    start=(ci == 0 and nt == 0), stop=False)
                        nc.tensor.matmul(
                            et[:, cb * 256:(cb + 1) * 256],
                            lhsT=xsr[:, cb * 128:(cb + 1) * 128],
                            rhs=xsr[:], start=False, stop=True)
                    esb = pa.tile([128, 512], F32, tag="esb", name="esb")
                    nc.vector.tensor_copy(esb[:], et[:])
                    nc.sync.dma_start(rs_in[b // 2, t, b % 2], esb[:])

    # ---------------- Phase B: PAM chunks ----------
    with tc.tile_pool(name="pxff", bufs=1) as sbxf, \
            tc.tile_pool(name="pxf", bufs=2) as sbx, \
            tc.tile_pool(name="pX", bufs=1) as sbX, \
            tc.tile_pool(name="pqk", bufs=1) as sbqk, \
            tc.tile_pool(name="psmall", bufs=2) as sbs, \
            tc.tile_pool(name="psm", bufs=1) as sbm, \
            tc.tile_pool(name="qkps", bufs=1, space="PSUM") as psq, \
            tc.tile_pool(name="eps", bufs=1, space="PSUM") as pse, \
            tc.tile_pool(name="avps", bufs=1, space="PSUM") as psa, \
            tc.tile_pool(name="ops", bufs=2, space="PSUM") as pso, \
            tc.tile_pool(name="gps", bufs=1, space="PSUM") as psg:

        def pam_unit(u):
            ci, t = u // 2, u % 2
            # load f32 chunk data, convert once to bf16 for the PAM pipeline
            xff = sbxf.tile([128, 8192], F32, tag="xff", name="xff")
            for cb in range(2):
                nc.sync.dma_start(
                    xff[:, cb * 4096:(cb + 1) * 4096],
                    I["xc"][ci, t].rearrange("b cb p n -> cb p b n")[cb])
            xf = []
            for cb in range(2):
                xft = sbx.tile([128, 4096], BF16, tag=f"xf{cb}",
                               name=f"xf{cb}")
                if cb == 0:
                    nc.scalar.copy(xft[:], xff[:, 0:4096])
                else:
                    nc.vector.tensor_copy(xft[:], xff[:, 4096:8192])
                xf.append(xft)
            qtb = sbqk.tile([128, 1024], F32, tag="qtb", name="qtb")
            ktb = sbqk.tile([128, 1024], F32, tag="ktb", name="ktb")
            _emit_qk(nc, cst, psq, xf, qtb, ktb)
            # f32: the per-chunk CKA sits ~6e-4 from perfect correlation and
            # percent-shifts if the gram features are rounded to bf16
            X = sbX.tile([128, 8192], F32, tag="X", name="X")
            for b in range(16):
                w = b // 2
                rb = 32 * (w % 4)
                fo = (w // 4) * 512 + (b % 2) * 256
                op_ = _emit_pam_sample(
                    nc, cst, sbs, psa, pso,
                    qtb[rb:rb + 32, fo:fo + 256], ktb[rb:rb + 32, fo:fo + 256],
                    xf, b * 256, pse, row_base=rb)
                nc.vector.tensor_copy(
                    X.rearrange("p (mt d b2) -> p mt d b2", mt=2, b2=16)[:, :, :, b],
                    op_.rearrange("p (mt d) -> p mt d", mt=2))
            gp = psg.tile([128, 128], F32, tag="gp", name="gp")
            for s in range(64):
                nc.tensor.matmul(
                    gp[:], lhsT=X[:, s * 128:(s + 1) * 128],
                    rhs=X[:, s * 128:(s + 1) * 128],
                    start=(s == 0), stop=(s == 63))
            gps = sbs.tile([128, 128], F32, tag="gpsb", name="gpsb")
            nc.scalar.copy(gps[:], gp[:])
            nc.sync.dma_start(O["gpam"][ci, t], gps[:])

        if "b" in phases:
            pam_unit(0)

        # RS after unit 0's DMAs so its queue traffic doesn't stall B
        if "a" in phases:
            nc.gpsimd.collective_compute(
                "ReduceScatter", ADD,
                replica_groups=[list(range(NCORES))],
                ins=[rs_in.opt()], outs=[rs_out.opt()])

        for u in range(1 if "b" in phases else 0, 3 if "b" in phases else 0):
            pam_unit(u)

        # -------- softmax of own CAM energies + attn AllGather --------
        if "a" in phases:
            eo = sbm.tile([128, 2048], F32, tag="eo", name="eo")
            for t in range(2):
                for bo in range(2):
                    nc.sync.dma_start(
                        eo[:, (t * 2 + bo) * 512:(t * 2 + bo + 1) * 512],
                        rs_out[t, bo])
            for t in range(2):
                for bo in range(2):
                    ecur = eo[:, (t * 2 + bo) * 512:(t * 2 + bo + 1) * 512]
                    # softmax of (min - E) rows == softmax(max_d E - E)
                    mn = sbm.tile([128, 2], F32, tag="mnc", name="mnc")
                    for cb in range(2):
                        nc.vector.tensor_reduce(
                            mn[:, cb:cb + 1],
                            ecur[:, cb * 256:(cb + 1) * 256], AX,
                            op=mybir.AluOpType.min)
                    expe = sbm.tile([128, 512], F32, tag="expec",
                                    name="expec")
                    zz = sbm.tile([128, 2], F32, tag="zzc", name="zzc")
                    for cb in range(2):
                        nc.scalar.activation(
                            expe[:, cb * 256:(cb + 1) * 256],
                            ecur[:, cb * 256:(cb + 1) * 256],
                            EXP, bias=mn[:, cb:cb + 1], scale=-1.0,
                            accum_out=zz[:, cb:cb + 1])
                    rr = sbm.tile([128, 2], F32, tag="rrc", name="rrc")
                    nc.vector.reciprocal(rr[:], zz[:])
                    dgc = sbm.tile([128, 256], F32, tag="dgc", name="dgc")
                    for cb in range(2):
                        nc.vector.tensor_scalar_mul(
                            dgc[:, cb * 128:(cb + 1) * 128],
                            cst["gicam"][:], rr[:, cb:cb + 1])
                    # reuse the PAM ep2 PSUM buffer (phase-B pool budget)
                    atc = pse.tile([128, 512], F32, tag="ep2", name="ep2")
                    for dt in range(2):
                        for cb in range(2):
                            nc.tensor.matmul(
                                atc[:, dt * 256 + cb * 128:
                                    dt * 256 + cb * 128 + 128],
                                lhsT=expe[:, cb * 256 + dt * 128:
                                          cb * 256 + dt * 128 + 128],
                                rhs=dgc[:, cb * 128:(cb + 1) * 128],
                                start=True, stop=True)
                    atcs = sbm.tile([128, 512], F32, tag="atcs",
                                    name="atcs")
                    nc.vector.tensor_copy(atcs[:], atc[:])
                    for dt in range(2):
                        nc.sync.dma_start(
                            atnb[bo * 4 + t * 2 + dt],
                            atcs[:, dt * 256:(dt + 1) * 256])
            nc.gpsimd.collective_compute(
                "AllGather", mybir.AluOpType.bypass,
                replica_groups=[list(range(NCORES))],
                ins=[atnb.opt()], outs=[atng.opt()])

        for u in range(3 if "b" in phases else 0, 6 if "b" in phases else 0):
            pam_unit(u)

        # chunk 24: 2 own samples, R^T straight to DRAM
        for t in range(2 if "b" in phases else 0):
            for bo in range(2):
                xf4 = []
                for cb in range(2):
                    x4 = sbs.tile([128, 256], BF16, tag=f"xf4{cb}",
                                  name=f"xf4{cb}")
                    nc.sync.dma_start(x4[:], I["x24o"][t, bo, cb])
                    xf4.append(x4)
                qtb4 = sbs.tile([32, 256], F32, tag="qtb4", name="qtb4")
                ktb4 = sbs.tile([32, 256], F32, tag="ktb4", name="ktb4")
                for which, wt, dst in (("q", "wqT", qtb4), ("k", "wkT", ktb4)):
                    qp4 = psq.tile([128, 1024], F32, tag="qkp", name="qkp")
                    for kb in range(2):
                        nc.tensor.matmul(
                            qp4[0:32, 0:256], lhsT=cst[wt][kb][:],
                            rhs=xf4[kb][:], start=(kb == 0), stop=(kb == 1))
                    if which == "q":
                        nc.scalar.activation(dst[:], qp4[0:32, 0:256], IDN,
                                             bias=cst["bq4"][0:32, :],
                                             scale=1.0)
                    else:
                        nc.scalar.copy(dst[:], qp4[0:32, 0:256])
                op4 = _emit_pam_sample(
                    nc, cst, sbs, psa, pso, qtb4[:], ktb4[:],
                    xf4, 0, pse)
                op4s = sbs.tile([128, 512], F32, tag="op4s", name="op4s")
                nc.vector.tensor_copy(op4s[:], op4[:])
                for mt in range(2):
                    nc.sync.dma_start(
                        O["c24r"][t, bo, mt],
                        op4s[:, mt * 256:(mt + 1) * 256])

    # ---------------- Phase C: CAM out-slice + partial grams ----------
    with tc.tile_pool(name="c2at", bufs=1) as sb2a, \
            tc.tile_pool(name="c2x", bufs=1) as sb2x, \
            tc.tile_pool(name="c2n", bufs=3) as sb2n, \
            tc.tile_pool(name="c2ops", bufs=2, space="PSUM") as ps2o, \
            tc.tile_pool(name="c2gps", bufs=1, space="PSUM") as ps2g:
        for t in range(2 if "c" in phases else 0):
            atn = sb2a.tile([128, 8192], F32, tag="atn", name="atn")
            for b in range(16):
                for dt in range(2):
                    nc.sync.dma_start(
                        atn[:, (b * 2 + dt) * 256:(b * 2 + dt + 1) * 256],
                        atng[b // 2, (b % 2) * 4 + t * 2 + dt])
            gacc = sb2n.tile([128, 128], F32, tag="gacc", name="gacc")
            for ci in range(3):
                Xw = sb2x.tile([128, 8192], F32, tag="Xw", name="Xw")
                for b in range(16):
                    xn = sb2n.tile([128, 512], F32, tag="xn2", name="xn2")
                    for cb in range(2):
                        nc.sync.dma_start(
                            xn[:, cb * 256:(cb + 1) * 256],
                            I["xc"][ci, t, b, cb])
                    ocp = ps2o.tile([128, 1024], F32, tag="ocp", name="ocp")
                    for cb in range(2):
                        for dt in range(2):
                            nc.tensor.matmul(
                                ocp[:, cb * 512:cb * 512 + 256],
                                lhsT=atn[:, (b * 2 + dt) * 256 + cb * 128:
                                         (b * 2 + dt) * 256 + cb * 128 + 128],
                                rhs=xn[:, dt * 256:(dt + 1) * 256],
                                start=(dt == 0), stop=False)
                        nc.tensor.matmul(
                            ocp[:, cb * 512:cb * 512 + 256],
                            lhsT=cst["i128"][:],
                            rhs=xn[:, cb * 256:(cb + 1) * 256],
                            start=False, stop=True)
                    dst = Xw.rearrange(
                        "p (cb n b2) -> p cb n b2", cb=2, b2=16)[:, :, :, b]
                    src = ocp.rearrange("p (cb n) -> p cb n", cb=2)[:, :, 0:256]
                    if b % 2 == 0:
                        nc.vector.tensor_copy(dst, src)
                    else:
                        nc.scalar.copy(dst, src)
                gcp = ps2g.tile([128, 128], F32, tag="gcp", name="gcp")
                for s in range(64):
                    nc.tensor.matmul(
                        gcp[:], lhsT=Xw[:, s * 128:(s + 1) * 128],
                        rhs=Xw[:, s * 128:(s + 1) * 128],
                        start=(s == 0), stop=(s == 63))
                if ci == 0:
                    nc.vector.tensor_copy(gacc[:], gcp[:])
                else:
                    nc.vector.tensor_tensor(gacc[:], gacc[:], gcp[:], op=ADD)
            # chunk-24 position-slice window
            Xs = sb2x.tile([128, 1024], F32, tag="Xs", name="Xs")
            for b in range(16):
                xs = sb2n.tile([128, 64], F32, tag="xs2", name="xs2")
                for cb in range(2):
                    nc.sync.dma_start(
                        xs[:, cb * 32:(cb + 1) * 32], I["x24s"][t, b, cb])
                oc2 = ps2o.tile([128, 64], F32, tag="oc2", name="oc2")
                for cb in range(2):
                    for dt in range(2):
                        nc.tensor.matmul(
                            oc2[:, cb * 32:(cb + 1) * 32],
                            lhsT=atn[:, (b * 2 + dt) * 256 + cb * 128:
                                     (b * 2 + dt) * 256 + cb * 128 + 128],
                            rhs=xs[:, dt * 32:(dt + 1) * 32],
                            start=(dt == 0), stop=False)
                    nc.tensor.matmul(
                        oc2[:, cb * 32:(cb + 1) * 32],
                        lhsT=cst["i128"][:],
                        rhs=xs[:, cb * 32:(cb + 1) * 32],
                        start=False, stop=True)
                dst = Xs.rearrange(
                    "p (cb ns b2) -> p cb ns b2", cb=2, b2=16)[:, :, :, b]
                src = oc2.rearrange("p (cb ns) -> p cb ns", cb=2)
                if b % 2 == 0:
                    nc.vector.tensor_copy(dst, src)
                else:
                    nc.scalar.copy(dst, src)
            gcp = ps2g.tile([128, 128], F32, tag="gcp", name="gcp")
            for s in range(8):
                nc.tensor.matmul(
                    gcp[:], lhsT=Xs[:, s * 128:(s + 1) * 128],
                    rhs=Xs[:, s * 128:(s + 1) * 128],
                    start=(s == 0), stop=(s == 7))
            nc.vector.tensor_tensor(gacc[:], gacc[:], gcp[:], op=ADD)
            gcs = sb2n.tile([128, 128], F32, tag="gcs", name="gcs")
            nc.scalar.copy(gcs[:], gacc[:])
            nc.sync.dma_start(O["gcam"][t], gcs[:])


_PROG = None


def _get_prog():
    global _PROG
    if _PROG is None:
        nc = bacc.Bacc("TRN2", target_bir_lowering=False, debug=False,
                       num_devices=NCORES)
        I = {n: nc.dram_tensor(n, list(s[0]), s[1], kind="ExternalInput").ap()
             for n, s in IN_SPECS.items()}
        O = {n: nc.dram_tensor(n, list(s), F32, kind="ExternalOutput").ap()
             for n, s in OUT_SPECS.items()}
        _emit_program(nc, I, O)
        nc.compile()
        _PROG = nc
    return _PROG


# --------------------------------------------------------------------------
# host side
# --------------------------------------------------------------------------

def _make_in_maps(feat_S, feat_T, Wq, bq, Wk, bk, Wv, bv, gammacam, gammapam):
    gp = float(np.asarray(gammapam).reshape(-1)[0])
    gc = float(np.asarray(gammacam).reshape(-1)[0])
    gbv = (gp * np.asarray(bv, np.float32)).astype(np.float32)
    BF = ml_dtypes.bfloat16

    # chunk-major global rearrange: [25, 2, 16, 2, 128, 256] f32
    A = np.empty((25, 2, 16, 256, 256), np.float32)
    for t, X in enumerate((feat_S, feat_T)):
        A[:, t] = (np.asarray(X, np.float32)
                   .reshape(B, C, 5, 16, 5, 16)
                   .transpose(2, 4, 0, 1, 3, 5)
                   .reshape(25, B, C, 256))
    A = A.reshape(25, 2, 16, 2, 128, 256)

    consts = {
        "wqT": np.ascontiguousarray(
            np.asarray(Wq, np.float32).T.reshape(2, 128, CK)).astype(BF),
        "wkT": np.ascontiguousarray(
            np.asarray(Wk, np.float32).T.reshape(2, 128, CK)).astype(BF),
        "wvT": np.ascontiguousarray(
            (gp * np.asarray(Wv, np.float32)).T.reshape(2, 128, C)).astype(BF),
        "bq4": np.ascontiguousarray(np.tile(np.asarray(bq, np.float32), 4)[:, None]),
        "i128": np.eye(128, dtype=np.float32),
        "i128b": np.eye(128, dtype=np.float32).astype(BF),
        "gicam": (gc * np.eye(128)).astype(np.float32),
        "gbv512": np.ascontiguousarray(np.tile(gbv, (128, 2))),
    }

    in_maps = []
    for j in range(NCORES):
        m = dict(consts)
        m["xc"] = A[3 * j:3 * j + 3]
        m["x24o"] = np.ascontiguousarray(A[24][:, 2 * j:2 * j + 2]).astype(BF)
        m["x24s"] = np.ascontiguousarray(A[24][:, :, :, :, 32 * j:32 * j + 32])
        in_maps.append(m)
    return in_maps


def _diag16(gfull):
    """gfull: [..., 128, 128] partials; f64-sum partials then diagonal blocks."""
    gf = gfull.astype(np.float64).reshape(-1, 128, 128).sum(axis=0)
    g = np.zeros((16, 16), np.float64)
    for r in range(8):
        g += gf[16 * r:16 * r + 16, 16 * r:16 * r + 16]
    return g


def _cka_loss(KS, KT):
    def cgram(K):
        rm = K.mean(axis=1, keepdims=True)
        cm = K.mean(axis=0, keepdims=True)
        return K - rm - cm + K.mean()
    cX, cY = cgram(KS), cgram(KT)
    hsic = float((cX * cY).sum())
    v1 = float(np.sqrt((cX * cX).sum()))
    v2 = float(np.sqrt((cY * cY).sum()))
    return -np.log(np.abs(hsic / (v1 * v2)) + 1e-8)


def _postprocess(results):
    losses = []
    for c in range(24):
        j, ci = divmod(c, 3)
        res = results[j]
        KS = _diag16(res["gpam"][ci, 0])
        KT = _diag16(res["gpam"][ci, 1])
        losses.append(_cka_loss(KS, KT))
    # chunk 24 on host
    FS = np.empty((B, 2 * 128 * 256), np.float32)
    FT = np.empty((B, 2 * 128 * 256), np.float32)
    for j in range(NCORES):
        for bo in range(2):
            FS[2 * j + bo] = results[j]["c24r"][0, bo].reshape(-1)
            FT[2 * j + bo] = results[j]["c24r"][1, bo].reshape(-1)
    FS = FS.astype(np.float64)
    FT = FT.astype(np.float64)
    KS24 = FS @ FS.T
    KT24 = FT @ FT.T
    losses.append(_cka_loss(KS24, KT24))
    loss_PAM = float(np.mean(losses))

    KSc = np.zeros((16, 16), np.float64)
    KTc = np.zeros((16, 16), np.float64)
    for j in range(NCORES):
        KSc += _diag16(results[j]["gcam"][0])
        KTc += _diag16(results[j]["gcam"][1])
    loss_CAM = float(_cka_loss(KSc, KTc))
    return np.float32(loss_CAM), np.float32(loss_PAM)


def _run_sim(nc, in_maps):
    from concourse.bass_interp import MultiCoreSim
    sim = MultiCoreSim(nc, num_cores=NCORES)
    cores = list(sim.cores.values())
    for j, core in enumerate(cores):
        for name, arr in in_maps[j].items():
            core.tensor(name)[:] = arr
    sim.simulate()
    return [{n: core.tensor(n).copy() for n in OUT_SPECS} for core in cores]


_LAST_EXEC_NS = None


def kernel(**inputs):
    global _LAST_EXEC_NS
    nc = _get_prog()
    in_maps = _make_in_maps(**{k: np.asarray(v) for k, v in inputs.items()})
    if os.environ.get("CRIT_BACKEND", "hw") == "sim":
        results = _run_sim(nc, in_maps)
    else:
        res = bass_utils.run_bass_kernel_spmd(
            nc, in_maps, core_ids=list(range(NCORES)),
            trace=os.environ.get("CRIT_TRACE", "0") == "1")
        results = res.results
        _LAST_EXEC_NS = res.exec_time_ns
    return _postprocess(results)


# revision 40
# speedup vs baseline: 1.4046x; 1.1528x over previous
"""Bass/Trainium2 kernel for nn_CriterionSA (CAM/gridPAM CKA loss).

Self-contained: hardcodes shapes/sharding for the
B=16, C=256, H=W=80 problem on 8 NeuronCores.

Sharding (v3 — chunk-sharded bf16 shipping, ~13MB/core host->device):
  Raw features are shipped once, in bf16, grid-chunk partitioned:
    - xc:   core j owns grid chunks 3j..3j+2 in natural [C,N] layout for
            ALL 16 samples, both tensors.
    - x24o: chunk 24 for the core's own 2 samples (PAM chunk-24 is
            sample-split).
    - x24s: chunk 24, positions 32j..32j+32, ALL samples (CAM spatial
            coverage).
  On-device repartitioning:
    - X^T tiles come from DMA crossbar transposes (bf16); per-sample [C,C]
      CAM energy partials accumulate in PSUM (f32) over the core's
      positions, staged per-sample into an 8MB ReduceScatter(add) that
      hands each core the full energies of its 2 own samples.
    - CAM attention rows (f32) are AllGathered (1MB/core); each core then
      computes the CAM output over its 800 spatial positions for all 16
      samples (f32) and a partial [128,128] gram.
  PAM runs fully in bf16 (inputs/q/k/v/attention/supers) with f32 PSUM;
  the gamma*bv bias is folded into the v^T copy (attention rows sum to 1).
  CAM energy/attention/output stay f32 (softmax exponent sensitivity).
"""

import os
import sys

import numpy as np

_REPO = "/opt/trn_rl_repo"
if _REPO not in sys.path:
    sys.path.insert(0, _REPO)

import ml_dtypes
import concourse.bacc as bacc
import concourse.mybir as mybir
import concourse.tile as tile
from concourse import bass_utils

F32 = mybir.dt.float32
BF16 = mybir.dt.bfloat16
EXP = mybir.ActivationFunctionType.Exp
IDN = mybir.ActivationFunctionType.Identity
AX = mybir.AxisListType.X
ADD = mybir.AluOpType.add

NCORES = 8
B, C, H, W = 16, 256, 80, 80
CK = 32          # C // 8
TAU = 1.0

IN_SPECS = {
    # feature shards: f32 for the CAM paths (the CAM CKA loss sits at
    # correlation-distance ~5e-5 from 1 and percent-shifts under bf16
    # feature rounding); PAM-only data ships bf16.
    "xc":   ((3, 2, 16, 2, 128, 256), F32),   # (ci, t, b, cb, c_low, n)
    "x24o": ((2, 2, 2, 128, 256), BF16),      # (t, own-b, cb, c_low, n)
    "x24s": ((2, 16, 2, 128, 32), F32),       # (t, b, cb, c_low, ns)
    # weights / constants
    "wqT":  ((2, 128, 32), BF16),
    "wkT":  ((2, 128, 32), BF16),
    "wvT":  ((2, 128, 256), BF16),            # (gamma_pam * Wv)^T
    "bq4":  ((128, 1), F32),
    "i128": ((128, 128), F32),
    "i128b": ((128, 128), BF16),
    "gicam": ((128, 128), F32),               # gamma_cam * I
    "gbv512": ((128, 512), F32),              # gamma_pam*bv bcast (2x 256)
}
OUT_SPECS = {
    "gpam": (3, 2, 128, 128),         # per (ci, t) chunk gram supers
    "gcam": (2, 128, 128),            # per t CAM gram partial
    "c24r": (2, 2, 2, 128, 256),      # (t, own-b, m-tile, m_low, c) PAM R^T
}


# --------------------------------------------------------------------------
# device program
# --------------------------------------------------------------------------

def _emit_softmax_attn_T(nc, sb, ep, eye_ap, n_i, tag):
    """From energy PSUM tile ep [128, 512] (two 256-wide row-blocks along
    free), produce (expE f32 [128,512], dg f32 [128,256]) where dg holds two
    128x128 diagonal blocks diag(1/Z). Softmax rows are the PARTITION dim of
    each 256-block; normalization uses exp(E - rowmax). Kept in f32 so the
    normalized attention is only rounded to bf16 once (at the av copy)."""
    nm = sb.tile([128, 2], F32, tag=f"nm{tag}", name=f"nm{tag}")
    nc.vector.tensor_reduce(
        nm[:], ep.rearrange("p (i j) -> p i j", i=2), AX,
        op=mybir.AluOpType.max, negate=True)
    expe = sb.tile([128, 512], F32, tag=f"expe{tag}", name=f"expe{tag}")
    zz = sb.tile([128, 2], F32, tag=f"zz{tag}", name=f"zz{tag}")
    for i in range(n_i):
        nc.scalar.activation(
            expe[:, i * 256:(i + 1) * 256], ep[:, i * 256:(i + 1) * 256],
            EXP, bias=nm[:, i:i + 1], scale=1.0, accum_out=zz[:, i:i + 1])
    rr = sb.tile([128, 2], F32, tag=f"rr{tag}", name=f"rr{tag}")
    nc.vector.reciprocal(rr[:, 0:n_i], zz[:, 0:n_i])
    dg = sb.tile([128, 256], F32, tag=f"dg{tag}", name=f"dg{tag}")
    for i in range(n_i):
        nc.vector.tensor_scalar_mul(
            dg[:, i * 128:(i + 1) * 128], eye_ap, rr[:, i:i + 1])
    return expe, dg


def _emit_pam_sample(nc, cst, sbs, psa, pso, q_sl, k_sl, xf, boff,
                     ep2_pool, row_base=0):
    """One PAM attention sample (bf16 pipeline, f32 PSUM). q_sl/k_sl:
    [32,256] bf16 APs at base partition row_base. xf: 2 natural bf16 c-tiles;
    boff: free offset of this sample in xf. The residual X^T comes from
    TensorE transposes of the xf blocks; gamma*bv is folded into the v^T
    copy (attention rows sum to 1).
    Returns op_ PSUM tile [128, 512] = R^T, layout (m-tile 2)(c 256)."""
    ep2 = ep2_pool.tile([128, 512], F32, tag="ep2", name="ep2")
    for ib in range(2):
        nc.tensor.matmul(
            ep2[:, ib * 256:(ib + 1) * 256],
            lhsT=q_sl[:, ib * 128:(ib + 1) * 128], rhs=k_sl,
            start=True, stop=True, tile_position=(row_base, 0))
    expe, dg = _emit_softmax_attn_T(nc, sbs, ep2, cst["i128"][:], 2, "p")
    avp = psa.tile([128, 1024], F32, tag="avp", name="avp")
    # A^T (normalized) blocks: avp[:, jb*256+ib*128] = expE[ib-rows, jb-cols]^T * diag
    for jb in range(2):
        for ib in range(2):
            nc.tensor.matmul(
                avp[:, jb * 256 + ib * 128: jb * 256 + ib * 128 + 128],
                lhsT=expe[:, ib * 256 + jb * 128: ib * 256 + jb * 128 + 128],
                rhs=dg[:, ib * 128:(ib + 1) * 128], start=True, stop=True)
    # v^T = Xf^T @ (gamma Wv)^T
    for jb in range(2):
        for cb in range(2):
            nc.tensor.matmul(
                avp[:, 512 + jb * 256: 512 + (jb + 1) * 256],
                lhsT=xf[cb][:, boff + jb * 128: boff + jb * 128 + 128],
                rhs=cst["wvT"][cb][:], start=(cb == 0), stop=(cb == 1))
    av = sbs.tile([128, 1024], BF16, tag="av", name="av")
    nc.scalar.copy(av[:, 0:512], avp[:, 0:512])
    # v^T + gamma*bv (rows of attention sum to 1, so the bias folds here)
    nc.vector.tensor_tensor(
        av[:, 512:1024], avp[:, 512:1024], cst["gbv512"][:], op=ADD)
    op_ = pso.tile([128, 512], F32, tag="opam", name="opam")
    for mb in range(2):
        for jb in range(2):
            nc.tensor.matmul(
                op_[:, mb * 256:(mb + 1) * 256],
                lhsT=av[:, jb * 256 + mb * 128: jb * 256 + mb * 128 + 128],
                rhs=av[:, 512 + jb * 256: 512 + (jb + 1) * 256],
                start=(jb == 0), stop=False)
        # residual: += X^T (transpose of xf m-block)
        for cb in range(2):
            nc.tensor.matmul(
                op_[:, mb * 256 + cb * 128: mb * 256 + cb * 128 + 128],
                lhsT=xf[cb][:, boff + mb * 128: boff + mb * 128 + 128],
                rhs=cst["i128b"][:], start=False, stop=(cb == 1))
    return op_


def _emit_qk(nc, cst, psq, xf, qtb, ktb):
    """q/k passes over a 16-sample chunk unit (samples col-packed 4-wide)."""
    for which, wt, dst in (("q", "wqT", qtb), ("k", "wkT", ktb)):
        qp = psq.tile([128, 1024], F32, tag="qkp", name="qkp")
        for w in range(8):
            r_ = 32 * (w % 4)
            fo = (w // 4) * 512
            for kb in range(2):
                nc.tensor.matmul(
                    qp[r_:r_ + 32, fo:fo + 512],
                    lhsT=cst[wt][kb][:],
                    rhs=xf[kb][:, w * 512:(w + 1) * 512],
                    start=(kb == 0), stop=(kb == 1),
                    tile_position=(0, r_))
        if which == "q":
            nc.scalar.activation(dst[:], qp[:], IDN,
                                 bias=cst["bq4"][:], scale=1.0)
        else:
            nc.scalar.copy(dst[:], qp[:])


def _emit_program(nc, I, O):
    phases = os.environ.get("CRIT_PHASES", "abc")
    with tile.TileContext(nc) as tc:
        cpool = tc.alloc_tile_pool(name="const", bufs=1)
        dram = tc.alloc_tile_pool(name="ccdram", bufs=1, space="DRAM")
        cst = {}
        for nm_ in ("wqT", "wkT", "wvT"):
            cst[nm_] = []
            for kb in range(2):
                t = cpool.tile(list(IN_SPECS[nm_][0][1:]), IN_SPECS[nm_][1],
                               name=f"{nm_}{kb}")
                nc.sync.dma_start(t[:], I[nm_][kb])
                cst[nm_].append(t)
        for nm_ in ("bq4", "i128", "i128b", "gicam", "gbv512"):
            t = cpool.tile(list(IN_SPECS[nm_][0]), IN_SPECS[nm_][1], name=nm_)
            nc.sync.dma_start(t[:], I[nm_][:])
            cst[nm_] = t

        # (sh, t, bo, p, (cb d)) — partition-major rows, f32 energies
        rs_in = dram.tile([8, 2, 2, 128, 512], F32, name="rs_in")
        rs_out = dram.tile([2, 2, 128, 512], F32, name="rs_out")
        atnb = dram.tile([8, 128, 256], F32, name="atnb")
        atng = dram.tile([8, 8, 128, 256], F32, name="atng", addr_space="Shared")

        for _rep in range(int(os.environ.get("CRIT_REPS", "1"))):
            _emit_body(tc, nc, I, O, cst, rs_in, rs_out, atnb, atng, phases)

        cpool.release()
        dram.release()


def _emit_body(tc, nc, I, O, cst, rs_in, rs_out, atnb, atng, phases):
    # ---------------- Phase A: XBAR transposes + energy partials ----------
    if "a" in phases:
        with tc.tile_pool(name="pa", bufs=3) as pa, \
                tc.tile_pool(name="paT", bufs=2, space="PSUM") as psT, \
                tc.tile_pool(name="paE", bufs=2, space="PSUM") as psE:
            for t in range(2):
                for b in range(16):
                    xtrs = []
                    for ci in range(3):
                        xn = pa.tile([128, 512], F32, tag=f"xn{ci}",
                                     name=f"xn{ci}")
                        for cb in range(2):
                            nc.sync.dma_start(
                                xn[:, cb * 256:(cb + 1) * 256],
                                I["xc"][ci, t, b, cb])
                        tp = psT.tile([128, 512], F32, tag="tp", name="tp")
                        for nt in range(2):
                            for cb in range(2):
                                nc.tensor.matmul(
                                    tp[:, nt * 256 + cb * 128:
                                       nt * 256 + cb * 128 + 128],
                                    lhsT=xn[:, cb * 256 + nt * 128:
                                            cb * 256 + nt * 128 + 128],
                                    rhs=cst["i128"][:], start=True, stop=True)
                        xtr = pa.tile([128, 512], F32, tag=f"xtr{ci}",
                                      name=f"xtr{ci}")
                        nc.scalar.copy(xtr[:], tp[:])
                        xtrs.append(xtr)
                    # chunk-24 position slice (TensorE transpose, 32 pos)
                    xs = pa.tile([128, 64], F32, tag="xs", name="xs")
                    for cb in range(2):
                        nc.sync.dma_start(
                            xs[:, cb * 32:(cb + 1) * 32], I["x24s"][t, b, cb])
                    tps = psT.tile([32, 256], F32, tag="tps", name="tps")
                    for cb in range(2):
                        nc.tensor.matmul(
                            tps[:, cb * 128:(cb + 1) * 128],
                            lhsT=xs[:, cb * 32:(cb + 1) * 32],
                            rhs=cst["i128"][:], start=True, stop=True)
                    xsr = pa.tile([32, 256], F32, tag="xsr", name="xsr")
                    nc.vector.tensor_copy(xsr[:], tps[:])
                    # energy accumulation: one window's chain at a time
                    # (PSUM start zeroes the whole bank)
                    et = psE.tile([128, 512], F32, tag="et", name="et")
                    for cb in range(2):
                        for ci in range(3):
                            for nt in range(2):
                                nc.tensor.matmul(
                                    et[:, cb * 256:(cb + 1) * 256],
                                    lhsT=xtrs[ci][:, nt * 256 + cb * 128:
                                                  nt * 256 + cb * 128 + 128],
                                    rhs=xtrs[ci][:, nt * 256:(nt + 1) * 256],
                                    start=(ci == 0 and nt == 0), stop=False)
                        nc.tensor.matmul(
                            et[:, cb * 256:(cb + 1) * 256],
                            lhsT=xsr[:, cb * 128:(cb + 1) * 128],
                            rhs=xsr[:], start=False, stop=True)
                    esb = pa.tile([128, 512], F32, tag="esb", name="esb")
                    nc.vector.tensor_copy(esb[:], et[:])
                    nc.sync.dma_start(rs_in[b // 2, t, b % 2], esb[:])

    # ---------------- Phase B: PAM chunks ----------
    with tc.tile_pool(name="pxff", bufs=2) as sbxf, \
            tc.tile_pool(name="pxf", bufs=2) as sbx, \
            tc.tile_pool(name="pX", bufs=1) as sbX, \
            tc.tile_pool(name="pqk", bufs=1) as sbqk, \
            tc.tile_pool(name="psmall", bufs=2) as sbs, \
            tc.tile_pool(name="psm", bufs=1) as sbm, \
            tc.tile_pool(name="qkps", bufs=1, space="PSUM") as psq, \
            tc.tile_pool(name="eps", bufs=1, space="PSUM") as pse, \
            tc.tile_pool(name="avps", bufs=1, space="PSUM") as psa, \
            tc.tile_pool(name="ops", bufs=2, space="PSUM") as pso, \
            tc.tile_pool(name="gps", bufs=1, space="PSUM") as psg:

        def pam_unit(u):
            ci, t = u // 2, u % 2
            # load f32 chunk data, convert once to bf16 for the PAM pipeline
            xff = sbxf.tile([128, 8192], F32, tag="xff", name="xff")
            for cb in range(2):
                nc.sync.dma_start(
                    xff[:, cb * 4096:(cb + 1) * 4096],
                    I["xc"][ci, t].rearrange("b cb p n -> cb p b n")[cb])
            xf = []
            for cb in range(2):
                xft = sbx.tile([128, 4096], BF16, tag=f"xf{cb}",
                               name=f"xf{cb}")
                if cb == 0:
                    nc.scalar.copy(xft[:], xff[:, 0:4096])
                else:
                    nc.vector.tensor_copy(xft[:], xff[:, 4096:8192])
                xf.append(xft)
            qtb = sbqk.tile([128, 1024], F32, tag="qtb", name="qtb")
            ktb = sbqk.tile([128, 1024], F32, tag="ktb", name="ktb")
            _emit_qk(nc, cst, psq, xf, qtb, ktb)
            # f32: the per-chunk CKA sits ~6e-4 from perfect correlation and
            # percent-shifts if the gram features are rounded to bf16
            X = sbX.tile([128, 8192], F32, tag="X", name="X")
            for b in range(16):
                w = b // 2
                rb = 32 * (w % 4)
                fo = (w // 4) * 512 + (b % 2) * 256
                op_ = _emit_pam_sample(
                    nc, cst, sbs, psa, pso,
                    qtb[rb:rb + 32, fo:fo + 256], ktb[rb:rb + 32, fo:fo + 256],
                    xf, b * 256, pse, row_base=rb)
                nc.vector.tensor_copy(
                    X.rearrange("p (mt d b2) -> p mt d b2", mt=2, b2=16)[:, :, :, b],
                    op_.rearrange("p (mt d) -> p mt d", mt=2))
            gp = psg.tile([128, 128], F32, tag="gp", name="gp")
            for s in range(64):
                nc.tensor.matmul(
                    gp[:], lhsT=X[:, s * 128:(s + 1) * 128],
                    rhs=X[:, s * 128:(s + 1) * 128],
                    start=(s == 0), stop=(s == 63))
            gps = sbs.tile([128, 128], F32, tag="gpsb", name="gpsb")
            nc.scalar.copy(gps[:], gp[:])
            nc.sync.dma_start(O["gpam"][ci, t], gps[:])

        if "b" in phases:
            pam_unit(0)

        # RS after unit 0's DMAs so its queue traffic doesn't stall B
        if "a" in phases:
            nc.gpsimd.collective_compute(
                "ReduceScatter", ADD,
                replica_groups=[list(range(NCORES))],
                ins=[rs_in.opt()], outs=[rs_out.opt()])

        for u in range(1 if "b" in phases else 0, 3 if "b" in phases else 0):
            pam_unit(u)

        # -------- softmax of own CAM energies + attn AllGather --------
        if "a" in phases:
            eo = sbm.tile([128, 2048], F32, tag="eo", name="eo")
            for t in range(2):
                for bo in range(2):
                    nc.sync.dma_start(
                        eo[:, (t * 2 + bo) * 512:(t * 2 + bo + 1) * 512],
                        rs_out[t, bo])
            for t in range(2):
                for bo in range(2):
                    ecur = eo[:, (t * 2 + bo) * 512:(t * 2 + bo + 1) * 512]
                    # softmax of (min - E) rows == softmax(max_d E - E)
                    mn = sbm.tile([128, 2], F32, tag="mnc", name="mnc")
                    for cb in range(2):
                        nc.vector.tensor_reduce(
                            mn[:, cb:cb + 1],
                            ecur[:, cb * 256:(cb + 1) * 256], AX,
                            op=mybir.AluOpType.min)
                    expe = sbm.tile([128, 512], F32, tag="expec",
                                    name="expec")
                    zz = sbm.tile([128, 2], F32, tag="zzc", name="zzc")
                    for cb in range(2):
                        nc.scalar.activation(
                            expe[:, cb * 256:(cb + 1) * 256],
                            ecur[:, cb * 256:(cb + 1) * 256],
                            EXP, bias=mn[:, cb:cb + 1], scale=-1.0,
                            accum_out=zz[:, cb:cb + 1])
                    rr = sbm.tile([128, 2], F32, tag="rrc", name="rrc")
                    nc.vector.reciprocal(rr[:], zz[:])
                    dgc = sbm.tile([128, 256], F32, tag="dgc", name="dgc")
                    for cb in range(2):
                        nc.vector.tensor_scalar_mul(
                            dgc[:, cb * 128:(cb + 1) * 128],
                            cst["gicam"][:], rr[:, cb:cb + 1])
                    # reuse the PAM ep2 PSUM buffer (phase-B pool budget)
                    atc = pse.tile([128, 512], F32, tag="ep2", name="ep2")
                    for dt in range(2):
                        for cb in range(2):
                            nc.tensor.matmul(
                                atc[:, dt * 256 + cb * 128:
                                    dt * 256 + cb * 128 + 128],
                                lhsT=expe[:, cb * 256 + dt * 128:
                                          cb * 256 + dt * 128 + 128],
                                rhs=dgc[:, cb * 128:(cb + 1) * 128],
                                start=True, stop=True)
                    atcs = sbm.tile([128, 512], F32, tag="atcs",
                                    name="atcs")
                    nc.vector.tensor_copy(atcs[:], atc[:])
                    for dt in range(2):
                        nc.sync.dma_start(
                            atnb[bo * 4 + t * 2 + dt],
                            atcs[:, dt * 256:(dt + 1) * 256])
            nc.gpsimd.collective_compute(
                "AllGather", mybir.AluOpType.bypass,
                replica_groups=[list(range(NCORES))],
                ins=[atnb.opt()], outs=[atng.opt()])

        for u in range(3 if "b" in phases else 0, 6 if "b" in phases else 0):
            pam_unit(u)

        # chunk 24: 2 own samples, R^T straight to DRAM
        for t in range(2 if "b" in phases else 0):
            for bo in range(2):
                xf4 = []
                for cb in range(2):
                    x4 = sbs.tile([128, 256], BF16, tag=f"xf4{cb}",
                                  name=f"xf4{cb}")
                    nc.sync.dma_start(x4[:], I["x24o"][t, bo, cb])
                    xf4.append(x4)
                qtb4 = sbs.tile([32, 256], F32, tag="qtb4", name="qtb4")
                ktb4 = sbs.tile([32, 256], F32, tag="ktb4", name="ktb4")
                for which, wt, dst in (("q", "wqT", qtb4), ("k", "wkT", ktb4)):
                    qp4 = psq.tile([128, 1024], F32, tag="qkp", name="qkp")
                    for kb in range(2):
                        nc.tensor.matmul(
                            qp4[0:32, 0:256], lhsT=cst[wt][kb][:],
                            rhs=xf4[kb][:], start=(kb == 0), stop=(kb == 1))
                    if which == "q":
                        nc.scalar.activation(dst[:], qp4[0:32, 0:256], IDN,
                                             bias=cst["bq4"][0:32, :],
                                             scale=1.0)
                    else:
                        nc.scalar.copy(dst[:], qp4[0:32, 0:256])
                op4 = _emit_pam_sample(
                    nc, cst, sbs, psa, pso, qtb4[:], ktb4[:],
                    xf4, 0, pse)
                op4s = sbs.tile([128, 512], F32, tag="op4s", name="op4s")
                nc.vector.tensor_copy(op4s[:], op4[:])
                for mt in range(2):
                    nc.sync.dma_start(
                        O["c24r"][t, bo, mt],
                        op4s[:, mt * 256:(mt + 1) * 256])

    # ---------------- Phase C: CAM out-slice + partial grams ----------
    # All 800 of the core's positions (3 chunks + 32 c24-slice) are packed
    # into one [128, (cb, 800)] tile per (t,b); attention matmuls run on two
    # 400-wide windows and the +X residual rides the supers copy as a
    # tensor_tensor add (no identity matmuls).
    with tc.tile_pool(name="c2at", bufs=1) as sb2a, \
            tc.tile_pool(name="c2x", bufs=1) as sb2x, \
            tc.tile_pool(name="c2n", bufs=3) as sb2n, \
            tc.tile_pool(name="c2ops", bufs=2, space="PSUM") as ps2o, \
            tc.tile_pool(name="c2gps", bufs=1, space="PSUM") as ps2g:
        for t in range(2 if "c" in phases else 0):
            atn = sb2a.tile([128, 8192], F32, tag="atn", name="atn")
            for b in range(16):
                for dt in range(2):
                    nc.sync.dma_start(
                        atn[:, (b * 2 + dt) * 256:(b * 2 + dt + 1) * 256],
                        atng[b // 2, (b % 2) * 4 + t * 2 + dt])
            gacc = sb2n.tile([128, 128], F32, tag="gacc", name="gacc")
            Xws = [sb2x.tile([128, 12800], F32, tag=f"Xw{w}", name=f"Xw{w}")
                   for w in range(2)]
            for b in range(16):
                xa = sb2n.tile([128, 1600], F32, tag="xa", name="xa")
                for cb in range(2):
                    for ci in range(3):
                        nc.sync.dma_start(
                            xa[:, cb * 800 + ci * 256: cb * 800 + ci * 256 + 256],
                            I["xc"][ci, t, b, cb])
                    nc.sync.dma_start(
                        xa[:, cb * 800 + 768: cb * 800 + 800],
                        I["x24s"][t, b, cb])
                for w in range(2):
                    ocp = ps2o.tile([128, 1024], F32, tag="ocp", name="ocp")
                    for cb in range(2):
                        for dt in range(2):
                            nc.tensor.matmul(
                                ocp[:, cb * 512:cb * 512 + 400],
                                lhsT=atn[:, (b * 2 + dt) * 256 + cb * 128:
                                         (b * 2 + dt) * 256 + cb * 128 + 128],
                                rhs=xa[:, dt * 800 + w * 400:
                                       dt * 800 + w * 400 + 400],
                                start=(dt == 0), stop=(dt == 1))
                    dst = Xws[w].rearrange(
                        "p (cb n b2) -> p cb n b2", cb=2, b2=16)[:, :, :, b]
                    src = ocp.rearrange("p (cb n) -> p cb n", cb=2)[:, :, 0:400]
                    res = xa.rearrange(
                        "p (cb n) -> p cb n", cb=2)[:, :, w * 400:w * 400 + 400]
                    nc.vector.tensor_tensor(dst, src, res, op=ADD)
            for w in range(2):
                gcp = ps2g.tile([128, 128], F32, tag="gcp", name="gcp")
                for s in range(100):
                    nc.tensor.matmul(
                        gcp[:], lhsT=Xws[w][:, s * 128:(s + 1) * 128],
                        rhs=Xws[w][:, s * 128:(s + 1) * 128],
                        start=(s == 0), stop=(s == 99))
                if w == 0:
                    nc.vector.tensor_copy(gacc[:], gcp[:])
                else:
                    nc.vector.tensor_tensor(gacc[:], gacc[:], gcp[:], op=ADD)
            gcs = sb2n.tile([128, 128], F32, tag="gcs", name="gcs")
            nc.scalar.copy(gcs[:], gacc[:])
            nc.sync.dma_start(O["gcam"][t], gcs[:])


_PROG = None


def _get_prog():
    global _PROG
    if _PROG is None:
        nc = bacc.Bacc("TRN2", target_bir_lowering=False, debug=False,
                       num_devices=NCORES)
        I = {n: nc.dram_tensor(n, list(s[0]), s[1], kind="ExternalInput").ap()
             for n, s in IN_SPECS.items()}
        O = {n: nc.dram_tensor(n, list(s), F32, kind="ExternalOutput").ap()
             for n, s in OUT_SPECS.items()}
        _emit_program(nc, I, O)
        nc.compile()
        _PROG = nc
    return _PROG


# --------------------------------------------------------------------------
# host side
# --------------------------------------------------------------------------

def _make_in_maps(feat_S, feat_T, Wq, bq, Wk, bk, Wv, bv, gammacam, gammapam):
    gp = float(np.asarray(gammapam).reshape(-1)[0])
    gc = float(np.asarray(gammacam).reshape(-1)[0])
    gbv = (gp * np.asarray(bv, np.float32)).astype(np.float32)
    BF = ml_dtypes.bfloat16

    # chunk-major global rearrange: [25, 2, 16, 2, 128, 256] f32
    A = np.empty((25, 2, 16, 256, 256), np.float32)
    for t, X in enumerate((feat_S, feat_T)):
        A[:, t] = (np.asarray(X, np.float32)
                   .reshape(B, C, 5, 16, 5, 16)
                   .transpose(2, 4, 0, 1, 3, 5)
                   .reshape(25, B, C, 256))
    A = A.reshape(25, 2, 16, 2, 128, 256)

    consts = {
        "wqT": np.ascontiguousarray(
            np.asarray(Wq, np.float32).T.reshape(2, 128, CK)).astype(BF),
        "wkT": np.ascontiguousarray(
            np.asarray(Wk, np.float32).T.reshape(2, 128, CK)).astype(BF),
        "wvT": np.ascontiguousarray(
            (gp * np.asarray(Wv, np.float32)).T.reshape(2, 128, C)).astype(BF),
        "bq4": np.ascontiguousarray(np.tile(np.asarray(bq, np.float32), 4)[:, None]),
        "i128": np.eye(128, dtype=np.float32),
        "i128b": np.eye(128, dtype=np.float32).astype(BF),
        "gicam": (gc * np.eye(128)).astype(np.float32),
        "gbv512": np.ascontiguousarray(np.tile(gbv, (128, 2))),
    }

    in_maps = []
    for j in range(NCORES):
        m = dict(consts)
        m["xc"] = A[3 * j:3 * j + 3]
        m["x24o"] = np.ascontiguousarray(A[24][:, 2 * j:2 * j + 2]).astype(BF)
        m["x24s"] = np.ascontiguousarray(A[24][:, :, :, :, 32 * j:32 * j + 32])
        in_maps.append(m)
    return in_maps


def _diag16(gfull):
    """gfull: [..., 128, 128] partials; f64-sum partials then diagonal blocks."""
    gf = gfull.astype(np.float64).reshape(-1, 128, 128).sum(axis=0)
    g = np.zeros((16, 16), np.float64)
    for r in range(8):
        g += gf[16 * r:16 * r + 16, 16 * r:16 * r + 16]
    return g


def _cka_loss(KS, KT):
    def cgram(K):
        rm = K.mean(axis=1, keepdims=True)
        cm = K.mean(axis=0, keepdims=True)
        return K - rm - cm + K.mean()
    cX, cY = cgram(KS), cgram(KT)
    hsic = float((cX * cY).sum())
    v1 = float(np.sqrt((cX * cX).sum()))
    v2 = float(np.sqrt((cY * cY).sum()))
    return -np.log(np.abs(hsic / (v1 * v2)) + 1e-8)


def _postprocess(results):
    losses = []
    for c in range(24):
        j, ci = divmod(c, 3)
        res = results[j]
        KS = _diag16(res["gpam"][ci, 0])
        KT = _diag16(res["gpam"][ci, 1])
        losses.append(_cka_loss(KS, KT))
    # chunk 24 on host
    FS = np.empty((B, 2 * 128 * 256), np.float32)
    FT = np.empty((B, 2 * 128 * 256), np.float32)
    for j in range(NCORES):
        for bo in range(2):
            FS[2 * j + bo] = results[j]["c24r"][0, bo].reshape(-1)
            FT[2 * j + bo] = results[j]["c24r"][1, bo].reshape(-1)
    FS = FS.astype(np.float64)
    FT = FT.astype(np.float64)
    KS24 = FS @ FS.T
    KT24 = FT @ FT.T
    losses.append(_cka_loss(KS24, KT24))
    loss_PAM = float(np.mean(losses))

    KSc = np.zeros((16, 16), np.float64)
    KTc = np.zeros((16, 16), np.float64)
    for j in range(NCORES):
        KSc += _diag16(results[j]["gcam"][0])
        KTc += _diag16(results[j]["gcam"][1])
    loss_CAM = float(_cka_loss(KSc, KTc))
    return np.float32(loss_CAM), np.float32(loss_PAM)


def _run_sim(nc, in_maps):
    from concourse.bass_interp import MultiCoreSim
    sim = MultiCoreSim(nc, num_cores=NCORES)
    cores = list(sim.cores.values())
    for j, core in enumerate(cores):
        for name, arr in in_maps[j].items():
            core.tensor(name)[:] = arr
    sim.simulate()
    return [{n: core.tensor(n).copy() for n in OUT_SPECS} for core in cores]


_LAST_EXEC_NS = None


def kernel(**inputs):
    global _LAST_EXEC_NS
    nc = _get_prog()
    in_maps = _make_in_maps(**{k: np.asarray(v) for k, v in inputs.items()})
    if os.environ.get("CRIT_BACKEND", "hw") == "sim":
        results = _run_sim(nc, in_maps)
    else:
        res = bass_utils.run_bass_kernel_spmd(
            nc, in_maps, core_ids=list(range(NCORES)),
            trace=os.environ.get("CRIT_TRACE", "0") == "1")
        results = res.results
        _LAST_EXEC_NS = res.exec_time_ns
    return _postprocess(results)


# revision 44
# speedup vs baseline: 1.4720x; 1.0480x over previous
"""Bass/Trainium2 kernel for nn_CriterionSA (CAM/gridPAM CKA loss).

Self-contained: hardcodes shapes/sharding for the
B=16, C=256, H=W=80 problem on 8 NeuronCores.

Sharding (v3 — chunk-sharded bf16 shipping, ~13MB/core host->device):
  Raw features are shipped once, in bf16, grid-chunk partitioned:
    - xc:   core j owns grid chunks 3j..3j+2 in natural [C,N] layout for
            ALL 16 samples, both tensors.
    - x24o: chunk 24 for the core's own 2 samples (PAM chunk-24 is
            sample-split).
    - x24s: chunk 24, positions 32j..32j+32, ALL samples (CAM spatial
            coverage).
  On-device repartitioning:
    - X^T tiles come from DMA crossbar transposes (bf16); per-sample [C,C]
      CAM energy partials accumulate in PSUM (f32) over the core's
      positions, staged per-sample into an 8MB ReduceScatter(add) that
      hands each core the full energies of its 2 own samples.
    - CAM attention rows (f32) are AllGathered (1MB/core); each core then
      computes the CAM output over its 800 spatial positions for all 16
      samples (f32) and a partial [128,128] gram.
  PAM runs fully in bf16 (inputs/q/k/v/attention/supers) with f32 PSUM;
  the gamma*bv bias is folded into the v^T copy (attention rows sum to 1).
  CAM energy/attention/output stay f32 (softmax exponent sensitivity).
"""

import os
import sys

import numpy as np

_REPO = "/opt/trn_rl_repo"
if _REPO not in sys.path:
    sys.path.insert(0, _REPO)

import ml_dtypes
import concourse.bacc as bacc
import concourse.mybir as mybir
import concourse.tile as tile
from concourse import bass_utils

F32 = mybir.dt.float32
BF16 = mybir.dt.bfloat16
EXP = mybir.ActivationFunctionType.Exp
IDN = mybir.ActivationFunctionType.Identity
AX = mybir.AxisListType.X
ADD = mybir.AluOpType.add

NCORES = 8
B, C, H, W = 16, 256, 80, 80
CK = 32          # C // 8
TAU = 1.0

IN_SPECS = {
    # feature shards: f32 for the CAM paths (the CAM CKA loss sits at
    # correlation-distance ~5e-5 from 1 and percent-shifts under bf16
    # feature rounding); PAM-only data ships bf16.
    "xc":   ((3, 2, 16, 2, 128, 256), F32),   # (ci, t, b, cb, c_low, n)
    "x24o": ((2, 2, 2, 128, 256), BF16),      # (t, own-b, cb, c_low, n)
    "x24s": ((2, 16, 2, 128, 32), F32),       # (t, b, cb, c_low, ns)
    # weights / constants
    "wqT":  ((2, 128, 32), BF16),
    "wkT":  ((2, 128, 32), BF16),
    "wvT":  ((2, 128, 256), BF16),            # (gamma_pam * Wv)^T
    "bq4":  ((128, 1), F32),
    "i128": ((128, 128), F32),
    "i128b": ((128, 128), BF16),
    "gicam": ((128, 128), F32),               # gamma_cam * I
    "gbv512": ((128, 512), F32),              # gamma_pam*bv bcast (2x 256)
}
OUT_SPECS = {
    "gpam": (3, 2, 128, 128),         # per (ci, t) chunk gram supers
    "gcam": (2, 128, 128),            # per t CAM gram partial
    "c24r": (2, 2, 2, 128, 256),      # (t, own-b, m-tile, m_low, c) PAM R^T
}


# --------------------------------------------------------------------------
# device program
# --------------------------------------------------------------------------

def _emit_softmax_attn_T(nc, sb, ep, eye_ap, n_i, tag):
    """From energy PSUM tile ep [128, 512] (two 256-wide row-blocks along
    free), produce (expE f32 [128,512], dg f32 [128,256]) where dg holds two
    128x128 diagonal blocks diag(1/Z). Softmax rows are the PARTITION dim of
    each 256-block; normalization uses exp(E - rowmax). Kept in f32 so the
    normalized attention is only rounded to bf16 once (at the av copy)."""
    nm = sb.tile([128, 2], F32, tag=f"nm{tag}", name=f"nm{tag}")
    nc.vector.tensor_reduce(
        nm[:], ep.rearrange("p (i j) -> p i j", i=2), AX,
        op=mybir.AluOpType.max, negate=True)
    expe = sb.tile([128, 512], BF16, tag=f"expe{tag}", name=f"expe{tag}")
    zz = sb.tile([128, 2], F32, tag=f"zz{tag}", name=f"zz{tag}")
    for i in range(n_i):
        nc.scalar.activation(
            expe[:, i * 256:(i + 1) * 256], ep[:, i * 256:(i + 1) * 256],
            EXP, bias=nm[:, i:i + 1], scale=1.0, accum_out=zz[:, i:i + 1])
    rr = sb.tile([128, 2], F32, tag=f"rr{tag}", name=f"rr{tag}")
    nc.vector.reciprocal(rr[:, 0:n_i], zz[:, 0:n_i])
    dg = sb.tile([128, 256], BF16, tag=f"dg{tag}", name=f"dg{tag}")
    for i in range(n_i):
        nc.vector.tensor_scalar_mul(
            dg[:, i * 128:(i + 1) * 128], eye_ap, rr[:, i:i + 1])
    return expe, dg


def _emit_pam_sample(nc, cst, sbs, psa, pso, q_sl, k_sl, xf, boff,
                     ep2_pool, row_base=0):
    """One PAM attention sample (bf16 pipeline, f32 PSUM). q_sl/k_sl:
    [32,256] bf16 APs at base partition row_base. xf: 2 natural bf16 c-tiles;
    boff: free offset of this sample in xf. The residual X^T comes from
    TensorE transposes of the xf blocks; gamma*bv is folded into the v^T
    copy (attention rows sum to 1).
    Returns op_ PSUM tile [128, 512] = R^T, layout (m-tile 2)(c 256)."""
    ep2 = ep2_pool.tile([128, 512], F32, tag="ep2", name="ep2")
    for ib in range(2):
        nc.tensor.matmul(
            ep2[:, ib * 256:(ib + 1) * 256],
            lhsT=q_sl[:, ib * 128:(ib + 1) * 128], rhs=k_sl,
            start=True, stop=True, tile_position=(row_base, 0))
    expe, dg = _emit_softmax_attn_T(nc, sbs, ep2, cst["i128b"][:], 2, "p")
    avp = psa.tile([128, 1024], F32, tag="avp", name="avp")
    # A^T (normalized) blocks: avp[:, jb*256+ib*128] = expE[ib-rows, jb-cols]^T * diag
    for jb in range(2):
        for ib in range(2):
            nc.tensor.matmul(
                avp[:, jb * 256 + ib * 128: jb * 256 + ib * 128 + 128],
                lhsT=expe[:, ib * 256 + jb * 128: ib * 256 + jb * 128 + 128],
                rhs=dg[:, ib * 128:(ib + 1) * 128], start=True, stop=True)
    # v^T = Xf^T @ (gamma Wv)^T
    for jb in range(2):
        for cb in range(2):
            nc.tensor.matmul(
                avp[:, 512 + jb * 256: 512 + (jb + 1) * 256],
                lhsT=xf[cb][:, boff + jb * 128: boff + jb * 128 + 128],
                rhs=cst["wvT"][cb][:], start=(cb == 0), stop=(cb == 1))
    av = sbs.tile([128, 1024], BF16, tag="av", name="av")
    nc.scalar.copy(av[:, 0:512], avp[:, 0:512])
    # v^T + gamma*bv (rows of attention sum to 1, so the bias folds here)
    nc.vector.tensor_tensor(
        av[:, 512:1024], avp[:, 512:1024], cst["gbv512"][:], op=ADD)
    op_ = pso.tile([128, 512], F32, tag="opam", name="opam")
    for mb in range(2):
        for jb in range(2):
            nc.tensor.matmul(
                op_[:, mb * 256:(mb + 1) * 256],
                lhsT=av[:, jb * 256 + mb * 128: jb * 256 + mb * 128 + 128],
                rhs=av[:, 512 + jb * 256: 512 + (jb + 1) * 256],
                start=(jb == 0), stop=False)
        # residual: += X^T (transpose of xf m-block)
        for cb in range(2):
            nc.tensor.matmul(
                op_[:, mb * 256 + cb * 128: mb * 256 + cb * 128 + 128],
                lhsT=xf[cb][:, boff + mb * 128: boff + mb * 128 + 128],
                rhs=cst["i128b"][:], start=False, stop=(cb == 1))
    return op_


def _emit_qk(nc, cst, psq, xf, qtb, ktb):
    """q/k passes over a 16-sample chunk unit (samples col-packed 4-wide)."""
    for which, wt, dst in (("q", "wqT", qtb), ("k", "wkT", ktb)):
        qp = psq.tile([128, 1024], F32, tag="qkp", name="qkp")
        for w in range(8):
            r_ = 32 * (w % 4)
            fo = (w // 4) * 512
            for kb in range(2):
                nc.tensor.matmul(
                    qp[r_:r_ + 32, fo:fo + 512],
                    lhsT=cst[wt][kb][:],
                    rhs=xf[kb][:, w * 512:(w + 1) * 512],
                    start=(kb == 0), stop=(kb == 1),
                    tile_position=(0, r_))
        if which == "q":
            nc.scalar.activation(dst[:], qp[:], IDN,
                                 bias=cst["bq4"][:], scale=1.0)
        else:
            nc.scalar.copy(dst[:], qp[:])


def _emit_program(nc, I, O):
    phases = os.environ.get("CRIT_PHASES", "abc")
    with tile.TileContext(nc) as tc:
        cpool = tc.alloc_tile_pool(name="const", bufs=1)
        dram = tc.alloc_tile_pool(name="ccdram", bufs=1, space="DRAM")
        cst = {}
        for nm_ in ("wqT", "wkT", "wvT"):
            cst[nm_] = []
            for kb in range(2):
                t = cpool.tile(list(IN_SPECS[nm_][0][1:]), IN_SPECS[nm_][1],
                               name=f"{nm_}{kb}")
                nc.sync.dma_start(t[:], I[nm_][kb])
                cst[nm_].append(t)
        for nm_ in ("bq4", "i128", "i128b", "gicam", "gbv512"):
            t = cpool.tile(list(IN_SPECS[nm_][0]), IN_SPECS[nm_][1], name=nm_)
            nc.sync.dma_start(t[:], I[nm_][:])
            cst[nm_] = t

        # (sh, t, bo, p, (cb d)) — partition-major rows, f32 energies
        rs_in = dram.tile([8, 2, 2, 128, 512], F32, name="rs_in")
        rs_out = dram.tile([2, 2, 128, 512], F32, name="rs_out")
        atnb = dram.tile([8, 128, 256], F32, name="atnb")
        atng = dram.tile([8, 8, 128, 256], F32, name="atng", addr_space="Shared")

        for _rep in range(int(os.environ.get("CRIT_REPS", "1"))):
            _emit_body(tc, nc, I, O, cst, rs_in, rs_out, atnb, atng, phases)

        cpool.release()
        dram.release()


def _emit_body(tc, nc, I, O, cst, rs_in, rs_out, atnb, atng, phases):
    # ---------------- Phase A: XBAR transposes + energy partials ----------
    if "a" in phases:
        with tc.tile_pool(name="pa", bufs=3) as pa, \
                tc.tile_pool(name="paT", bufs=2, space="PSUM") as psT, \
                tc.tile_pool(name="paE", bufs=2, space="PSUM") as psE:
            for t in range(2):
                for b in range(16):
                    xtrs = []
                    for ci in range(3):
                        xn = pa.tile([128, 512], F32, tag=f"xn{ci}",
                                     name=f"xn{ci}")
                        for cb in range(2):
                            nc.sync.dma_start(
                                xn[:, cb * 256:(cb + 1) * 256],
                                I["xc"][ci, t, b, cb])
                        tp = psT.tile([128, 512], F32, tag="tp", name="tp")
                        for nt in range(2):
                            for cb in range(2):
                                nc.tensor.matmul(
                                    tp[:, nt * 256 + cb * 128:
                                       nt * 256 + cb * 128 + 128],
                                    lhsT=xn[:, cb * 256 + nt * 128:
                                            cb * 256 + nt * 128 + 128],
                                    rhs=cst["i128"][:], start=True, stop=True)
                        xtr = pa.tile([128, 512], F32, tag=f"xtr{ci}",
                                      name=f"xtr{ci}")
                        nc.scalar.copy(xtr[:], tp[:])
                        xtrs.append(xtr)
                    # chunk-24 position slice (TensorE transpose, 32 pos)
                    xs = pa.tile([128, 64], F32, tag="xs", name="xs")
                    for cb in range(2):
                        nc.sync.dma_start(
                            xs[:, cb * 32:(cb + 1) * 32], I["x24s"][t, b, cb])
                    tps = psT.tile([32, 256], F32, tag="tps", name="tps")
                    for cb in range(2):
                        nc.tensor.matmul(
                            tps[:, cb * 128:(cb + 1) * 128],
                            lhsT=xs[:, cb * 32:(cb + 1) * 32],
                            rhs=cst["i128"][:], start=True, stop=True)
                    xsr = pa.tile([32, 256], F32, tag="xsr", name="xsr")
                    nc.vector.tensor_copy(xsr[:], tps[:])
                    # energy accumulation: one window's chain at a time
                    # (PSUM start zeroes the whole bank)
                    et = psE.tile([128, 512], F32, tag="et", name="et")
                    for cb in range(2):
                        for ci in range(3):
                            for nt in range(2):
                                nc.tensor.matmul(
                                    et[:, cb * 256:(cb + 1) * 256],
                                    lhsT=xtrs[ci][:, nt * 256 + cb * 128:
                                                  nt * 256 + cb * 128 + 128],
                                    rhs=xtrs[ci][:, nt * 256:(nt + 1) * 256],
                                    start=(ci == 0 and nt == 0), stop=False)
                        nc.tensor.matmul(
                            et[:, cb * 256:(cb + 1) * 256],
                            lhsT=xsr[:, cb * 128:(cb + 1) * 128],
                            rhs=xsr[:], start=False, stop=True)
                    esb = pa.tile([128, 512], F32, tag="esb", name="esb")
                    nc.vector.tensor_copy(esb[:], et[:])
                    nc.sync.dma_start(rs_in[b // 2, t, b % 2], esb[:])

    # ---------------- Phase B: PAM chunks ----------
    with tc.tile_pool(name="pxff", bufs=2) as sbxf, \
            tc.tile_pool(name="pxf", bufs=2) as sbx, \
            tc.tile_pool(name="pX", bufs=1) as sbX, \
            tc.tile_pool(name="pqk", bufs=1) as sbqk, \
            tc.tile_pool(name="psmall", bufs=2) as sbs, \
            tc.tile_pool(name="psm", bufs=1) as sbm, \
            tc.tile_pool(name="qkps", bufs=1, space="PSUM") as psq, \
            tc.tile_pool(name="eps", bufs=1, space="PSUM") as pse, \
            tc.tile_pool(name="avps", bufs=1, space="PSUM") as psa, \
            tc.tile_pool(name="ops", bufs=2, space="PSUM") as pso, \
            tc.tile_pool(name="gps", bufs=1, space="PSUM") as psg:

        def pam_load(u):
            ci, t = u // 2, u % 2
            # load f32 chunk data; a later pam_compute converts it to bf16.
            # Loads are hoisted ahead of the collectives (DMA rings stall
            # while a collective is in flight).
            xff = sbxf.tile([128, 8192], F32, tag="xff", name="xff")
            for cb in range(2):
                nc.sync.dma_start(
                    xff[:, cb * 4096:(cb + 1) * 4096],
                    I["xc"][ci, t].rearrange("b cb p n -> cb p b n")[cb])
            return xff

        def pam_compute(u, xff):
            ci, t = u // 2, u % 2
            xf = []
            for cb in range(2):
                xft = sbx.tile([128, 4096], BF16, tag=f"xf{cb}",
                               name=f"xf{cb}")
                if cb == 0:
                    nc.scalar.copy(xft[:], xff[:, 0:4096])
                else:
                    nc.vector.tensor_copy(xft[:], xff[:, 4096:8192])
                xf.append(xft)
            qtb = sbqk.tile([128, 1024], F32, tag="qtb", name="qtb")
            ktb = sbqk.tile([128, 1024], F32, tag="ktb", name="ktb")
            _emit_qk(nc, cst, psq, xf, qtb, ktb)
            X = sbX.tile([128, 8192], BF16, tag="X", name="X")
            for b in range(16):
                w = b // 2
                rb = 32 * (w % 4)
                fo = (w // 4) * 512 + (b % 2) * 256
                op_ = _emit_pam_sample(
                    nc, cst, sbs, psa, pso,
                    qtb[rb:rb + 32, fo:fo + 256], ktb[rb:rb + 32, fo:fo + 256],
                    xf, b * 256, pse, row_base=rb)
                nc.vector.tensor_copy(
                    X.rearrange("p (mt d b2) -> p mt d b2", mt=2, b2=16)[:, :, :, b],
                    op_.rearrange("p (mt d) -> p mt d", mt=2))
            gp = psg.tile([128, 128], F32, tag="gp", name="gp")
            for s in range(64):
                nc.tensor.matmul(
                    gp[:], lhsT=X[:, s * 128:(s + 1) * 128],
                    rhs=X[:, s * 128:(s + 1) * 128],
                    start=(s == 0), stop=(s == 63))
            gps = sbs.tile([128, 128], F32, tag="gpsb", name="gpsb")
            nc.scalar.copy(gps[:], gp[:])
            nc.sync.dma_start(O["gpam"][ci, t], gps[:])

        xffs = {}
        if "b" in phases:
            xffs[0] = pam_load(0)
            xffs[1] = pam_load(1)
            pam_compute(0, xffs.pop(0))

        # RS once units 0+1 are staged in SBUF: compute(1) runs through it
        if "a" in phases:
            nc.gpsimd.collective_compute(
                "ReduceScatter", ADD,
                replica_groups=[list(range(NCORES))],
                ins=[rs_in.opt()], outs=[rs_out.opt()])

        if "b" in phases:
            pam_compute(1, xffs.pop(1))
            xffs[2] = pam_load(2)
            pam_compute(2, xffs.pop(2))
            xffs[3] = pam_load(3)

        # -------- softmax of own CAM energies + attn AllGather --------
        if "a" in phases:
            eo = sbm.tile([128, 2048], F32, tag="eo", name="eo")
            for t in range(2):
                for bo in range(2):
                    nc.sync.dma_start(
                        eo[:, (t * 2 + bo) * 512:(t * 2 + bo + 1) * 512],
                        rs_out[t, bo])
            for t in range(2):
                for bo in range(2):
                    ecur = eo[:, (t * 2 + bo) * 512:(t * 2 + bo + 1) * 512]
                    # softmax of (min - E) rows == softmax(max_d E - E)
                    mn = sbm.tile([128, 2], F32, tag="mnc", name="mnc")
                    for cb in range(2):
                        nc.vector.tensor_reduce(
                            mn[:, cb:cb + 1],
                            ecur[:, cb * 256:(cb + 1) * 256], AX,
                            op=mybir.AluOpType.min)
                    expe = sbm.tile([128, 512], F32, tag="expec",
                                    name="expec")
                    zz = sbm.tile([128, 2], F32, tag="zzc", name="zzc")
                    for cb in range(2):
                        nc.scalar.activation(
                            expe[:, cb * 256:(cb + 1) * 256],
                            ecur[:, cb * 256:(cb + 1) * 256],
                            EXP, bias=mn[:, cb:cb + 1], scale=-1.0,
                            accum_out=zz[:, cb:cb + 1])
                    rr = sbm.tile([128, 2], F32, tag="rrc", name="rrc")
                    nc.vector.reciprocal(rr[:], zz[:])
                    dgc = sbm.tile([128, 256], F32, tag="dgc", name="dgc")
                    for cb in range(2):
                        nc.vector.tensor_scalar_mul(
                            dgc[:, cb * 128:(cb + 1) * 128],
                            cst["gicam"][:], rr[:, cb:cb + 1])
                    # reuse the PAM ep2 PSUM buffer (phase-B pool budget)
                    atc = pse.tile([128, 512], F32, tag="ep2", name="ep2")
                    for dt in range(2):
                        for cb in range(2):
                            nc.tensor.matmul(
                                atc[:, dt * 256 + cb * 128:
                                    dt * 256 + cb * 128 + 128],
                                lhsT=expe[:, cb * 256 + dt * 128:
                                          cb * 256 + dt * 128 + 128],
                                rhs=dgc[:, cb * 128:(cb + 1) * 128],
                                start=True, stop=True)
                    atcs = sbm.tile([128, 512], F32, tag="atcs",
                                    name="atcs")
                    nc.vector.tensor_copy(atcs[:], atc[:])
                    for dt in range(2):
                        nc.sync.dma_start(
                            atnb[bo * 4 + t * 2 + dt],
                            atcs[:, dt * 256:(dt + 1) * 256])
            nc.gpsimd.collective_compute(
                "AllGather", mybir.AluOpType.bypass,
                replica_groups=[list(range(NCORES))],
                ins=[atnb.opt()], outs=[atng.opt()])

        if "b" in phases:
            pam_compute(3, xffs.pop(3))
            for u in (4, 5):
                xff_u = pam_load(u)
                pam_compute(u, xff_u)

        # chunk 24: 2 own samples, R^T straight to DRAM
        for t in range(2 if "b" in phases else 0):
            for bo in range(2):
                xf4 = []
                for cb in range(2):
                    x4 = sbs.tile([128, 256], BF16, tag=f"xf4{cb}",
                                  name=f"xf4{cb}")
                    nc.sync.dma_start(x4[:], I["x24o"][t, bo, cb])
                    xf4.append(x4)
                qtb4 = sbs.tile([32, 256], F32, tag="qtb4", name="qtb4")
                ktb4 = sbs.tile([32, 256], F32, tag="ktb4", name="ktb4")
                for which, wt, dst in (("q", "wqT", qtb4), ("k", "wkT", ktb4)):
                    qp4 = psq.tile([128, 1024], F32, tag="qkp", name="qkp")
                    for kb in range(2):
                        nc.tensor.matmul(
                            qp4[0:32, 0:256], lhsT=cst[wt][kb][:],
                            rhs=xf4[kb][:], start=(kb == 0), stop=(kb == 1))
                    if which == "q":
                        nc.scalar.activation(dst[:], qp4[0:32, 0:256], IDN,
                                             bias=cst["bq4"][0:32, :],
                                             scale=1.0)
                    else:
                        nc.scalar.copy(dst[:], qp4[0:32, 0:256])
                op4 = _emit_pam_sample(
                    nc, cst, sbs, psa, pso, qtb4[:], ktb4[:],
                    xf4, 0, pse)
                op4s = sbs.tile([128, 512], F32, tag="op4s", name="op4s")
                nc.vector.tensor_copy(op4s[:], op4[:])
                for mt in range(2):
                    nc.sync.dma_start(
                        O["c24r"][t, bo, mt],
                        op4s[:, mt * 256:(mt + 1) * 256])

    # ---------------- Phase C: CAM out-slice + partial grams ----------
    # All 800 of the core's positions (3 chunks + 32 c24-slice) are packed
    # into one [128, (cb, 800)] tile per (t,b); attention matmuls run on two
    # 400-wide windows and the +X residual rides the supers copy as a
    # tensor_tensor add (no identity matmuls).
    with tc.tile_pool(name="c2at", bufs=1) as sb2a, \
            tc.tile_pool(name="c2x", bufs=1) as sb2x, \
            tc.tile_pool(name="c2n", bufs=3) as sb2n, \
            tc.tile_pool(name="c2ops", bufs=2, space="PSUM") as ps2o, \
            tc.tile_pool(name="c2gps", bufs=1, space="PSUM") as ps2g:
        for t in range(2 if "c" in phases else 0):
            atn = sb2a.tile([128, 8192], F32, tag="atn", name="atn")
            for b in range(16):
                for dt in range(2):
                    nc.sync.dma_start(
                        atn[:, (b * 2 + dt) * 256:(b * 2 + dt + 1) * 256],
                        atng[b // 2, (b % 2) * 4 + t * 2 + dt])
            gacc = sb2n.tile([128, 128], F32, tag="gacc", name="gacc")
            Xws = [sb2x.tile([128, 12800], F32, tag=f"Xw{w}", name=f"Xw{w}")
                   for w in range(2)]
            for b in range(16):
                xa = sb2n.tile([128, 1600], F32, tag="xa", name="xa")
                for cb in range(2):
                    for ci in range(3):
                        nc.sync.dma_start(
                            xa[:, cb * 800 + ci * 256: cb * 800 + ci * 256 + 256],
                            I["xc"][ci, t, b, cb])
                    nc.sync.dma_start(
                        xa[:, cb * 800 + 768: cb * 800 + 800],
                        I["x24s"][t, b, cb])
                for w in range(2):
                    ocp = ps2o.tile([128, 1024], F32, tag="ocp", name="ocp")
                    for cb in range(2):
                        for dt in range(2):
                            nc.tensor.matmul(
                                ocp[:, cb * 512:cb * 512 + 400],
                                lhsT=atn[:, (b * 2 + dt) * 256 + cb * 128:
                                         (b * 2 + dt) * 256 + cb * 128 + 128],
                                rhs=xa[:, dt * 800 + w * 400:
                                       dt * 800 + w * 400 + 400],
                                start=(dt == 0), stop=(dt == 1))
                    dst = Xws[w].rearrange(
                        "p (cb n b2) -> p cb n b2", cb=2, b2=16)[:, :, :, b]
                    src = ocp.rearrange("p (cb n) -> p cb n", cb=2)[:, :, 0:400]
                    res = xa.rearrange(
                        "p (cb n) -> p cb n", cb=2)[:, :, w * 400:w * 400 + 400]
                    nc.vector.tensor_tensor(dst, src, res, op=ADD)
            for w in range(2):
                gcp = ps2g.tile([128, 128], F32, tag="gcp", name="gcp")
                for s in range(100):
                    nc.tensor.matmul(
                        gcp[:], lhsT=Xws[w][:, s * 128:(s + 1) * 128],
                        rhs=Xws[w][:, s * 128:(s + 1) * 128],
                        start=(s == 0), stop=(s == 99))
                if w == 0:
                    nc.vector.tensor_copy(gacc[:], gcp[:])
                else:
                    nc.vector.tensor_tensor(gacc[:], gacc[:], gcp[:], op=ADD)
            gcs = sb2n.tile([128, 128], F32, tag="gcs", name="gcs")
            nc.scalar.copy(gcs[:], gacc[:])
            nc.sync.dma_start(O["gcam"][t], gcs[:])


_PROG = None


def _get_prog():
    global _PROG
    if _PROG is None:
        nc = bacc.Bacc("TRN2", target_bir_lowering=False, debug=False,
                       num_devices=NCORES)
        I = {n: nc.dram_tensor(n, list(s[0]), s[1], kind="ExternalInput").ap()
             for n, s in IN_SPECS.items()}
        O = {n: nc.dram_tensor(n, list(s), F32, kind="ExternalOutput").ap()
             for n, s in OUT_SPECS.items()}
        _emit_program(nc, I, O)
        nc.compile()
        _PROG = nc
    return _PROG


# --------------------------------------------------------------------------
# host side
# --------------------------------------------------------------------------

def _make_in_maps(feat_S, feat_T, Wq, bq, Wk, bk, Wv, bv, gammacam, gammapam):
    gp = float(np.asarray(gammapam).reshape(-1)[0])
    gc = float(np.asarray(gammacam).reshape(-1)[0])
    gbv = (gp * np.asarray(bv, np.float32)).astype(np.float32)
    BF = ml_dtypes.bfloat16

    # chunk-major global rearrange: [25, 2, 16, 2, 128, 256] f32
    A = np.empty((25, 2, 16, 256, 256), np.float32)
    for t, X in enumerate((feat_S, feat_T)):
        A[:, t] = (np.asarray(X, np.float32)
                   .reshape(B, C, 5, 16, 5, 16)
                   .transpose(2, 4, 0, 1, 3, 5)
                   .reshape(25, B, C, 256))
    A = A.reshape(25, 2, 16, 2, 128, 256)

    consts = {
        "wqT": np.ascontiguousarray(
            np.asarray(Wq, np.float32).T.reshape(2, 128, CK)).astype(BF),
        "wkT": np.ascontiguousarray(
            np.asarray(Wk, np.float32).T.reshape(2, 128, CK)).astype(BF),
        "wvT": np.ascontiguousarray(
            (gp * np.asarray(Wv, np.float32)).T.reshape(2, 128, C)).astype(BF),
        "bq4": np.ascontiguousarray(np.tile(np.asarray(bq, np.float32), 4)[:, None]),
        "i128": np.eye(128, dtype=np.float32),
        "i128b": np.eye(128, dtype=np.float32).astype(BF),
        "gicam": (gc * np.eye(128)).astype(np.float32),
        "gbv512": np.ascontiguousarray(np.tile(gbv, (128, 2))),
    }

    in_maps = []
    for j in range(NCORES):
        m = dict(consts)
        m["xc"] = A[3 * j:3 * j + 3]
        m["x24o"] = np.ascontiguousarray(A[24][:, 2 * j:2 * j + 2]).astype(BF)
        m["x24s"] = np.ascontiguousarray(A[24][:, :, :, :, 32 * j:32 * j + 32])
        in_maps.append(m)
    return in_maps


def _diag16(gfull):
    """gfull: [..., 128, 128] partials; f64-sum partials then diagonal blocks."""
    gf = gfull.astype(np.float64).reshape(-1, 128, 128).sum(axis=0)
    g = np.zeros((16, 16), np.float64)
    for r in range(8):
        g += gf[16 * r:16 * r + 16, 16 * r:16 * r + 16]
    return g


def _cka_loss(KS, KT):
    def cgram(K):
        rm = K.mean(axis=1, keepdims=True)
        cm = K.mean(axis=0, keepdims=True)
        return K - rm - cm + K.mean()
    cX, cY = cgram(KS), cgram(KT)
    hsic = float((cX * cY).sum())
    v1 = float(np.sqrt((cX * cX).sum()))
    v2 = float(np.sqrt((cY * cY).sum()))
    return -np.log(np.abs(hsic / (v1 * v2)) + 1e-8)


def _postprocess(results):
    losses = []
    for c in range(24):
        j, ci = divmod(c, 3)
        res = results[j]
        KS = _diag16(res["gpam"][ci, 0])
        KT = _diag16(res["gpam"][ci, 1])
        losses.append(_cka_loss(KS, KT))
    # chunk 24 on host
    FS = np.empty((B, 2 * 128 * 256), np.float32)
    FT = np.empty((B, 2 * 128 * 256), np.float32)
    for j in range(NCORES):
        for bo in range(2):
            FS[2 * j + bo] = results[j]["c24r"][0, bo].reshape(-1)
            FT[2 * j + bo] = results[j]["c24r"][1, bo].reshape(-1)
    FS = FS.astype(np.float64)
    FT = FT.astype(np.float64)
    KS24 = FS @ FS.T
    KT24 = FT @ FT.T
    losses.append(_cka_loss(KS24, KT24))
    loss_PAM = float(np.mean(losses))

    KSc = np.zeros((16, 16), np.float64)
    KTc = np.zeros((16, 16), np.float64)
    for j in range(NCORES):
        KSc += _diag16(results[j]["gcam"][0])
        KTc += _diag16(results[j]["gcam"][1])
    loss_CAM = float(_cka_loss(KSc, KTc))
    return np.float32(loss_CAM), np.float32(loss_PAM)


def _run_sim(nc, in_maps):
    from concourse.bass_interp import MultiCoreSim
    sim = MultiCoreSim(nc, num_cores=NCORES)
    cores = list(sim.cores.values())
    for j, core in enumerate(cores):
        for name, arr in in_maps[j].items():
            core.tensor(name)[:] = arr
    sim.simulate()
    return [{n: core.tensor(n).copy() for n in OUT_SPECS} for core in cores]


_LAST_EXEC_NS = None


def kernel(**inputs):
    global _LAST_EXEC_NS
    nc = _get_prog()
    in_maps = _make_in_maps(**{k: np.asarray(v) for k, v in inputs.items()})
    if os.environ.get("CRIT_BACKEND", "hw") == "sim":
        results = _run_sim(nc, in_maps)
    else:
        res = bass_utils.run_bass_kernel_spmd(
            nc, in_maps, core_ids=list(range(NCORES)),
            trace=os.environ.get("CRIT_TRACE", "0") == "1")
        results = res.results
        _LAST_EXEC_NS = res.exec_time_ns
    return _postprocess(results)


# revision 48
# speedup vs baseline: 1.9457x; 1.3218x over previous
"""Bass/Trainium2 kernel for nn_CriterionSA (CAM/gridPAM CKA loss).

Self-contained: hardcodes shapes/sharding for the
B=16, C=256, H=W=80 problem on 8 NeuronCores.

Sharding (v3 — chunk-sharded bf16 shipping, ~13MB/core host->device):
  Raw features are shipped once, in bf16, grid-chunk partitioned:
    - xc:   core j owns grid chunks 3j..3j+2 in natural [C,N] layout for
            ALL 16 samples, both tensors.
    - x24o: chunk 24 for the core's own 2 samples (PAM chunk-24 is
            sample-split).
    - x24s: chunk 24, positions 32j..32j+32, ALL samples (CAM spatial
            coverage).
  On-device repartitioning:
    - X^T tiles come from DMA crossbar transposes (bf16); per-sample [C,C]
      CAM energy partials accumulate in PSUM (f32) over the core's
      positions, staged per-sample into an 8MB ReduceScatter(add) that
      hands each core the full energies of its 2 own samples.
    - CAM attention rows (f32) are AllGathered (1MB/core); each core then
      computes the CAM output over its 800 spatial positions for all 16
      samples (f32) and a partial [128,128] gram.
  PAM runs fully in bf16 (inputs/q/k/v/attention/supers) with f32 PSUM;
  the gamma*bv bias is folded into the v^T copy (attention rows sum to 1).
  CAM energy/attention/output stay f32 (softmax exponent sensitivity).
"""

import os
import sys

import numpy as np

_REPO = "/opt/trn_rl_repo"
if _REPO not in sys.path:
    sys.path.insert(0, _REPO)

import ml_dtypes
import concourse.bacc as bacc
import concourse.mybir as mybir
import concourse.tile as tile
from concourse import bass_utils

F32 = mybir.dt.float32
F32R = mybir.dt.float32r
BF16 = mybir.dt.bfloat16
EXP = mybir.ActivationFunctionType.Exp
IDN = mybir.ActivationFunctionType.Identity
AX = mybir.AxisListType.X
ADD = mybir.AluOpType.add

NCORES = 8
B, C, H, W = 16, 256, 80, 80
CK = 32          # C // 8
TAU = 1.0

IN_SPECS = {
    # feature shards: f32 for the CAM paths (the CAM CKA loss sits at
    # correlation-distance ~5e-5 from 1 and percent-shifts under bf16
    # feature rounding); PAM-only data ships bf16.
    "xc":   ((3, 2, 16, 2, 128, 256), F32R),  # (ci, t, b, cb, c_low, n)
    "x24o": ((2, 2, 2, 128, 256), BF16),      # (t, own-b, cb, c_low, n)
    "x24s": ((2, 16, 2, 128, 32), F32R),      # (t, b, cb, c_low, ns)
    # weights / constants
    "wqT":  ((2, 128, 32), BF16),
    "wkT":  ((2, 128, 32), BF16),
    "wvT":  ((2, 128, 256), BF16),            # (gamma_pam * Wv)^T
    "bq4":  ((128, 1), F32),
    "i128": ((128, 128), F32),
    "i128b": ((128, 128), BF16),
    "i128r": ((128, 128), F32R),
    "gicam": ((128, 128), F32),               # gamma_cam * I
    "gbv512": ((128, 512), F32),              # gamma_pam*bv bcast (2x 256)
}
OUT_SPECS = {
    "gpam": (3, 2, 128, 128),         # per (ci, t) chunk gram supers
    "gcam": (2, 128, 128),            # per t CAM gram partial
    "c24r": (2, 2, 2, 128, 256),      # (t, own-b, m-tile, m_low, c) PAM R^T
}


# --------------------------------------------------------------------------
# device program
# --------------------------------------------------------------------------

def _emit_softmax_attn_T(nc, sb, ep, eye_ap, n_i, tag):
    """From energy PSUM tile ep [128, 512] (two 256-wide row-blocks along
    free), produce (expE f32 [128,512], dg f32 [128,256]) where dg holds two
    128x128 diagonal blocks diag(1/Z). Softmax rows are the PARTITION dim of
    each 256-block; normalization uses exp(E - rowmax). Kept in f32 so the
    normalized attention is only rounded to bf16 once (at the av copy)."""
    nm = sb.tile([128, 2], F32, tag=f"nm{tag}", name=f"nm{tag}")
    nc.vector.tensor_reduce(
        nm[:], ep.rearrange("p (i j) -> p i j", i=2), AX,
        op=mybir.AluOpType.max, negate=True)
    expe = sb.tile([128, 512], BF16, tag=f"expe{tag}", name=f"expe{tag}")
    zz = sb.tile([128, 2], F32, tag=f"zz{tag}", name=f"zz{tag}")
    for i in range(n_i):
        nc.scalar.activation(
            expe[:, i * 256:(i + 1) * 256], ep[:, i * 256:(i + 1) * 256],
            EXP, bias=nm[:, i:i + 1], scale=1.0, accum_out=zz[:, i:i + 1])
    rr = sb.tile([128, 2], F32, tag=f"rr{tag}", name=f"rr{tag}")
    nc.vector.reciprocal(rr[:, 0:n_i], zz[:, 0:n_i])
    dg = sb.tile([128, 256], BF16, tag=f"dg{tag}", name=f"dg{tag}")
    for i in range(n_i):
        nc.vector.tensor_scalar_mul(
            dg[:, i * 128:(i + 1) * 128], eye_ap, rr[:, i:i + 1])
    return expe, dg


def _emit_pam_sample(nc, cst, sbs, psa, pso, q_sl, k_sl, xf, boff,
                     ep2_pool, row_base=0):
    """One PAM attention sample (bf16 pipeline, f32 PSUM). q_sl/k_sl:
    [32,256] bf16 APs at base partition row_base. xf: 2 natural bf16 c-tiles;
    boff: free offset of this sample in xf. The residual X^T comes from
    TensorE transposes of the xf blocks; gamma*bv is folded into the v^T
    copy (attention rows sum to 1).
    Returns op_ PSUM tile [128, 512] = R^T, layout (m-tile 2)(c 256)."""
    ep2 = ep2_pool.tile([128, 512], F32, tag="ep2", name="ep2")
    for ib in range(2):
        nc.tensor.matmul(
            ep2[:, ib * 256:(ib + 1) * 256],
            lhsT=q_sl[:, ib * 128:(ib + 1) * 128], rhs=k_sl,
            start=True, stop=True, tile_position=(row_base, 0))
    expe, dg = _emit_softmax_attn_T(nc, sbs, ep2, cst["i128b"][:], 2, "p")
    avp = psa.tile([128, 1024], F32, tag="avp", name="avp")
    # A^T (normalized) blocks: avp[:, jb*256+ib*128] = expE[ib-rows, jb-cols]^T * diag
    for jb in range(2):
        for ib in range(2):
            nc.tensor.matmul(
                avp[:, jb * 256 + ib * 128: jb * 256 + ib * 128 + 128],
                lhsT=expe[:, ib * 256 + jb * 128: ib * 256 + jb * 128 + 128],
                rhs=dg[:, ib * 128:(ib + 1) * 128], start=True, stop=True)
    # v^T = Xf^T @ (gamma Wv)^T
    for jb in range(2):
        for cb in range(2):
            nc.tensor.matmul(
                avp[:, 512 + jb * 256: 512 + (jb + 1) * 256],
                lhsT=xf[cb][:, boff + jb * 128: boff + jb * 128 + 128],
                rhs=cst["wvT"][cb][:], start=(cb == 0), stop=(cb == 1))
    av = sbs.tile([128, 1024], BF16, tag="av", name="av")
    nc.scalar.copy(av[:, 0:512], avp[:, 0:512])
    # v^T + gamma*bv (rows of attention sum to 1, so the bias folds here)
    nc.vector.tensor_tensor(
        av[:, 512:1024], avp[:, 512:1024], cst["gbv512"][:], op=ADD)
    op_ = pso.tile([128, 512], F32, tag="opam", name="opam")
    for mb in range(2):
        for jb in range(2):
            nc.tensor.matmul(
                op_[:, mb * 256:(mb + 1) * 256],
                lhsT=av[:, jb * 256 + mb * 128: jb * 256 + mb * 128 + 128],
                rhs=av[:, 512 + jb * 256: 512 + (jb + 1) * 256],
                start=(jb == 0), stop=False)
        # residual: += X^T (transpose of xf m-block)
        for cb in range(2):
            nc.tensor.matmul(
                op_[:, mb * 256 + cb * 128: mb * 256 + cb * 128 + 128],
                lhsT=xf[cb][:, boff + mb * 128: boff + mb * 128 + 128],
                rhs=cst["i128b"][:], start=False, stop=(cb == 1))
    return op_


def _emit_qk(nc, cst, psq, xf, qtb, ktb):
    """q/k passes over a 16-sample chunk unit (samples col-packed 4-wide)."""
    for which, wt, dst in (("q", "wqT", qtb), ("k", "wkT", ktb)):
        qp = psq.tile([128, 1024], F32, tag="qkp", name="qkp")
        for w in range(8):
            r_ = 32 * (w % 4)
            fo = (w // 4) * 512
            for kb in range(2):
                nc.tensor.matmul(
                    qp[r_:r_ + 32, fo:fo + 512],
                    lhsT=cst[wt][kb][:],
                    rhs=xf[kb][:, w * 512:(w + 1) * 512],
                    start=(kb == 0), stop=(kb == 1),
                    tile_position=(0, r_))
        if which == "q":
            nc.scalar.activation(dst[:], qp[:], IDN,
                                 bias=cst["bq4"][:], scale=1.0)
        else:
            nc.scalar.copy(dst[:], qp[:])


def _emit_program(nc, I, O):
    phases = os.environ.get("CRIT_PHASES", "abc")
    with tile.TileContext(nc) as tc:
        cpool = tc.alloc_tile_pool(name="const", bufs=1)
        dram = tc.alloc_tile_pool(name="ccdram", bufs=1, space="DRAM")
        cst = {}
        for nm_ in ("wqT", "wkT", "wvT"):
            cst[nm_] = []
            for kb in range(2):
                t = cpool.tile(list(IN_SPECS[nm_][0][1:]), IN_SPECS[nm_][1],
                               name=f"{nm_}{kb}")
                nc.sync.dma_start(t[:], I[nm_][kb])
                cst[nm_].append(t)
        for nm_ in ("bq4", "i128", "i128b", "i128r", "gicam", "gbv512"):
            t = cpool.tile(list(IN_SPECS[nm_][0]), IN_SPECS[nm_][1], name=nm_)
            nc.sync.dma_start(t[:], I[nm_][:])
            cst[nm_] = t

        # (sh, t, bo, p, (cb d)) — partition-major rows, f32 energies
        rs_in = dram.tile([8, 2, 2, 128, 512], F32, name="rs_in")
        rs_out = dram.tile([2, 2, 128, 512], F32, name="rs_out")
        atnb = dram.tile([8, 128, 256], F32R, name="atnb")
        atng = dram.tile([8, 8, 128, 256], F32R, name="atng", addr_space="Shared")

        for _rep in range(int(os.environ.get("CRIT_REPS", "1"))):
            _emit_body(tc, nc, I, O, cst, rs_in, rs_out, atnb, atng, phases)

        cpool.release()
        dram.release()


def _emit_body(tc, nc, I, O, cst, rs_in, rs_out, atnb, atng, phases):
    with tc.tile_pool(name="pxff", bufs=2) as sbxf:
        _emit_body2(tc, nc, I, O, cst, rs_in, rs_out, atnb, atng, phases, sbxf)
    _emit_phase_c(tc, nc, I, O, cst, atng, phases)


def _emit_body2(tc, nc, I, O, cst, rs_in, rs_out, atnb, atng, phases, sbxf):
    def pam_load(u):
        ci, t = u // 2, u % 2
        # load f32 chunk data; a later pam_compute converts it to bf16.
        # The first two loads are hoisted before phase A so their transfers
        # complete before the ReduceScatter freezes the DMA rings.
        xff = sbxf.tile([128, 8192], F32R, tag="xff", name="xff")
        for cb in range(2):
            nc.sync.dma_start(
                xff[:, cb * 4096:(cb + 1) * 4096],
                I["xc"][ci, t].rearrange("b cb p n -> cb p b n")[cb])
        return xff

    xffs = {}
    if "b" in phases:
        xffs[0] = pam_load(0)
        xffs[1] = pam_load(1)

    # ---------------- Phase A: transposes + energy partials ----------
    if "a" in phases:
        with tc.tile_pool(name="pa", bufs=3) as pa, \
                tc.tile_pool(name="paT", bufs=2, space="PSUM") as psT, \
                tc.tile_pool(name="paE", bufs=2, space="PSUM") as psE:
            for t in range(2):
                for b in range(16):
                    xtrs = []
                    for ci in range(3):
                        xn = pa.tile([128, 512], F32R, tag=f"xn{ci}",
                                     name=f"xn{ci}")
                        for cb in range(2):
                            nc.sync.dma_start(
                                xn[:, cb * 256:(cb + 1) * 256],
                                I["xc"][ci, t, b, cb])
                        tp = psT.tile([128, 512], F32, tag="tp", name="tp")
                        for nt in range(2):
                            for cb in range(2):
                                nc.tensor.matmul(
                                    tp[:, nt * 256 + cb * 128:
                                       nt * 256 + cb * 128 + 128],
                                    lhsT=xn[:, cb * 256 + nt * 128:
                                            cb * 256 + nt * 128 + 128],
                                    rhs=cst["i128r"][:], start=True, stop=True)
                        xtr = pa.tile([128, 512], F32R, tag=f"xtr{ci}",
                                      name=f"xtr{ci}")
                        nc.scalar.copy(xtr[:], tp[:])
                        xtrs.append(xtr)
                    # chunk-24 position slice (TensorE transpose, 32 pos)
                    xs = pa.tile([128, 64], F32R, tag="xs", name="xs")
                    for cb in range(2):
                        nc.sync.dma_start(
                            xs[:, cb * 32:(cb + 1) * 32], I["x24s"][t, b, cb])
                    tps = psT.tile([32, 256], F32, tag="tps", name="tps")
                    for cb in range(2):
                        nc.tensor.matmul(
                            tps[:, cb * 128:(cb + 1) * 128],
                            lhsT=xs[:, cb * 32:(cb + 1) * 32],
                            rhs=cst["i128r"][:], start=True, stop=True)
                    xsr = pa.tile([32, 256], F32R, tag="xsr", name="xsr")
                    nc.vector.tensor_copy(xsr[:], tps[:])
                    # energy accumulation: one window's chain at a time
                    # (PSUM start zeroes the whole bank)
                    et = psE.tile([128, 512], F32, tag="et", name="et")
                    for cb in range(2):
                        for ci in range(3):
                            for nt in range(2):
                                nc.tensor.matmul(
                                    et[:, cb * 256:(cb + 1) * 256],
                                    lhsT=xtrs[ci][:, nt * 256 + cb * 128:
                                                  nt * 256 + cb * 128 + 128],
                                    rhs=xtrs[ci][:, nt * 256:(nt + 1) * 256],
                                    start=(ci == 0 and nt == 0), stop=False)
                        nc.tensor.matmul(
                            et[:, cb * 256:(cb + 1) * 256],
                            lhsT=xsr[:, cb * 128:(cb + 1) * 128],
                            rhs=xsr[:], start=False, stop=True)
                    esb = pa.tile([128, 512], F32, tag="esb", name="esb")
                    nc.vector.tensor_copy(esb[:], et[:])
                    nc.sync.dma_start(rs_in[b // 2, t, b % 2], esb[:])

    # ---------------- Phase B: PAM chunks ----------
    with tc.tile_pool(name="pxf", bufs=2) as sbx, \
            tc.tile_pool(name="pX", bufs=1) as sbX, \
            tc.tile_pool(name="pqk", bufs=1) as sbqk, \
            tc.tile_pool(name="psmall", bufs=2) as sbs, \
            tc.tile_pool(name="psm", bufs=1) as sbm, \
            tc.tile_pool(name="qkps", bufs=1, space="PSUM") as psq, \
            tc.tile_pool(name="eps", bufs=1, space="PSUM") as pse, \
            tc.tile_pool(name="avps", bufs=1, space="PSUM") as psa, \
            tc.tile_pool(name="ops", bufs=2, space="PSUM") as pso, \
            tc.tile_pool(name="gps", bufs=1, space="PSUM") as psg:

        def pam_compute(u, xff):
            ci, t = u // 2, u % 2
            xf = []
            for cb in range(2):
                xft = sbx.tile([128, 4096], BF16, tag=f"xf{cb}",
                               name=f"xf{cb}")
                if cb == 0:
                    nc.scalar.copy(xft[:], xff[:, 0:4096])
                else:
                    nc.vector.tensor_copy(xft[:], xff[:, 4096:8192])
                xf.append(xft)
            qtb = sbqk.tile([128, 1024], F32R, tag="qtb", name="qtb")
            ktb = sbqk.tile([128, 1024], F32R, tag="ktb", name="ktb")
            _emit_qk(nc, cst, psq, xf, qtb, ktb)
            X = sbX.tile([128, 8192], BF16, tag="X", name="X")
            for b in range(16):
                w = b // 2
                rb = 32 * (w % 4)
                fo = (w // 4) * 512 + (b % 2) * 256
                op_ = _emit_pam_sample(
                    nc, cst, sbs, psa, pso,
                    qtb[rb:rb + 32, fo:fo + 256], ktb[rb:rb + 32, fo:fo + 256],
                    xf, b * 256, pse, row_base=rb)
                nc.vector.tensor_copy(
                    X.rearrange("p (mt d b2) -> p mt d b2", mt=2, b2=16)[:, :, :, b],
                    op_.rearrange("p (mt d) -> p mt d", mt=2))
            gp = psg.tile([128, 128], F32, tag="gp", name="gp")
            for s in range(64):
                nc.tensor.matmul(
                    gp[:], lhsT=X[:, s * 128:(s + 1) * 128],
                    rhs=X[:, s * 128:(s + 1) * 128],
                    start=(s == 0), stop=(s == 63))
            gps = sbs.tile([128, 128], F32, tag="gpsb", name="gpsb")
            nc.scalar.copy(gps[:], gp[:])
            nc.sync.dma_start(O["gpam"][ci, t], gps[:])

        if "b" in phases:
            pam_compute(0, xffs.pop(0))

        # RS once units 0+1 are staged in SBUF: compute(1) runs through it
        if "a" in phases:
            nc.gpsimd.collective_compute(
                "ReduceScatter", ADD,
                replica_groups=[list(range(NCORES))],
                ins=[rs_in.opt()], outs=[rs_out.opt()])

        if "b" in phases:
            pam_compute(1, xffs.pop(1))
            xffs[2] = pam_load(2)
            pam_compute(2, xffs.pop(2))
            xffs[3] = pam_load(3)

        # -------- softmax of own CAM energies + attn AllGather --------
        if "a" in phases:
            eo = sbm.tile([128, 2048], F32, tag="eo", name="eo")
            for t in range(2):
                for bo in range(2):
                    nc.sync.dma_start(
                        eo[:, (t * 2 + bo) * 512:(t * 2 + bo + 1) * 512],
                        rs_out[t, bo])
            for t in range(2):
                for bo in range(2):
                    ecur = eo[:, (t * 2 + bo) * 512:(t * 2 + bo + 1) * 512]
                    # softmax of (min - E) rows == softmax(max_d E - E)
                    mn = sbm.tile([128, 2], F32, tag="mnc", name="mnc")
                    for cb in range(2):
                        nc.vector.tensor_reduce(
                            mn[:, cb:cb + 1],
                            ecur[:, cb * 256:(cb + 1) * 256], AX,
                            op=mybir.AluOpType.min)
                    expe = sbm.tile([128, 512], F32, tag="expec",
                                    name="expec")
                    zz = sbm.tile([128, 2], F32, tag="zzc", name="zzc")
                    for cb in range(2):
                        nc.scalar.activation(
                            expe[:, cb * 256:(cb + 1) * 256],
                            ecur[:, cb * 256:(cb + 1) * 256],
                            EXP, bias=mn[:, cb:cb + 1], scale=-1.0,
                            accum_out=zz[:, cb:cb + 1])
                    rr = sbm.tile([128, 2], F32, tag="rrc", name="rrc")
                    nc.vector.reciprocal(rr[:], zz[:])
                    dgc = sbm.tile([128, 256], F32, tag="dgc", name="dgc")
                    for cb in range(2):
                        nc.vector.tensor_scalar_mul(
                            dgc[:, cb * 128:(cb + 1) * 128],
                            cst["gicam"][:], rr[:, cb:cb + 1])
                    # reuse the PAM ep2 PSUM buffer (phase-B pool budget)
                    atc = pse.tile([128, 512], F32, tag="ep2", name="ep2")
                    for dt in range(2):
                        for cb in range(2):
                            nc.tensor.matmul(
                                atc[:, dt * 256 + cb * 128:
                                    dt * 256 + cb * 128 + 128],
                                lhsT=expe[:, cb * 256 + dt * 128:
                                          cb * 256 + dt * 128 + 128],
                                rhs=dgc[:, cb * 128:(cb + 1) * 128],
                                start=True, stop=True)
                    atcs = sbm.tile([128, 512], F32R, tag="atcs",
                                    name="atcs")
                    nc.vector.tensor_copy(atcs[:], atc[:])
                    for dt in range(2):
                        nc.sync.dma_start(
                            atnb[bo * 4 + t * 2 + dt],
                            atcs[:, dt * 256:(dt + 1) * 256])
            nc.gpsimd.collective_compute(
                "AllGather", mybir.AluOpType.bypass,
                replica_groups=[list(range(NCORES))],
                ins=[atnb.opt()], outs=[atng.opt()])

        if "b" in phases:
            pam_compute(3, xffs.pop(3))
            for u in (4, 5):
                xff_u = pam_load(u)
                pam_compute(u, xff_u)

        # chunk 24: 2 own samples, R^T straight to DRAM
        for t in range(2 if "b" in phases else 0):
            for bo in range(2):
                xf4 = []
                for cb in range(2):
                    x4 = sbs.tile([128, 256], BF16, tag=f"xf4{cb}",
                                  name=f"xf4{cb}")
                    nc.sync.dma_start(x4[:], I["x24o"][t, bo, cb])
                    xf4.append(x4)
                qtb4 = sbs.tile([32, 256], F32R, tag="qtb4", name="qtb4")
                ktb4 = sbs.tile([32, 256], F32R, tag="ktb4", name="ktb4")
                for which, wt, dst in (("q", "wqT", qtb4), ("k", "wkT", ktb4)):
                    qp4 = psq.tile([128, 1024], F32, tag="qkp", name="qkp")
                    for kb in range(2):
                        nc.tensor.matmul(
                            qp4[0:32, 0:256], lhsT=cst[wt][kb][:],
                            rhs=xf4[kb][:], start=(kb == 0), stop=(kb == 1))
                    if which == "q":
                        nc.scalar.activation(dst[:], qp4[0:32, 0:256], IDN,
                                             bias=cst["bq4"][0:32, :],
                                             scale=1.0)
                    else:
                        nc.scalar.copy(dst[:], qp4[0:32, 0:256])
                op4 = _emit_pam_sample(
                    nc, cst, sbs, psa, pso, qtb4[:], ktb4[:],
                    xf4, 0, pse)
                op4s = sbs.tile([128, 512], F32, tag="op4s", name="op4s")
                nc.vector.tensor_copy(op4s[:], op4[:])
                for mt in range(2):
                    nc.sync.dma_start(
                        O["c24r"][t, bo, mt],
                        op4s[:, mt * 256:(mt + 1) * 256])


def _emit_phase_c(tc, nc, I, O, cst, atng, phases):
    # ---------------- Phase C: CAM out-slice + partial grams ----------
    # All 800 of the core's positions (3 chunks + 32 c24-slice) are packed
    # into one [128, (cb, 800)] tile per (t,b); attention matmuls run on two
    # 400-wide windows and the +X residual rides the supers copy as a
    # tensor_tensor add (no identity matmuls).
    with tc.tile_pool(name="c2at", bufs=1) as sb2a, \
            tc.tile_pool(name="c2x", bufs=1) as sb2x, \
            tc.tile_pool(name="c2n", bufs=3) as sb2n, \
            tc.tile_pool(name="c2ops", bufs=2, space="PSUM") as ps2o, \
            tc.tile_pool(name="c2gps", bufs=1, space="PSUM") as ps2g:
        for t in range(2 if "c" in phases else 0):
            atn = sb2a.tile([128, 8192], F32R, tag="atn", name="atn")
            for b in range(16):
                for dt in range(2):
                    nc.sync.dma_start(
                        atn[:, (b * 2 + dt) * 256:(b * 2 + dt + 1) * 256],
                        atng[b // 2, (b % 2) * 4 + t * 2 + dt])
            gacc = sb2n.tile([128, 128], F32, tag="gacc", name="gacc")
            Xws = [sb2x.tile([128, 12800], F32, tag=f"Xw{w}", name=f"Xw{w}")
                   for w in range(2)]
            for b in range(16):
                xa = sb2n.tile([128, 1600], F32R, tag="xa", name="xa")
                for cb in range(2):
                    for ci in range(3):
                        nc.sync.dma_start(
                            xa[:, cb * 800 + ci * 256: cb * 800 + ci * 256 + 256],
                            I["xc"][ci, t, b, cb])
                    nc.sync.dma_start(
                        xa[:, cb * 800 + 768: cb * 800 + 800],
                        I["x24s"][t, b, cb])
                for w in range(2):
                    ocp = ps2o.tile([128, 1024], F32, tag="ocp", name="ocp")
                    for cb in range(2):
                        for dt in range(2):
                            nc.tensor.matmul(
                                ocp[:, cb * 512:cb * 512 + 400],
                                lhsT=atn[:, (b * 2 + dt) * 256 + cb * 128:
                                         (b * 2 + dt) * 256 + cb * 128 + 128],
                                rhs=xa[:, dt * 800 + w * 400:
                                       dt * 800 + w * 400 + 400],
                                start=(dt == 0), stop=(dt == 1))
                    dst = Xws[w].rearrange(
                        "p (cb n b2) -> p cb n b2", cb=2, b2=16)[:, :, :, b]
                    src = ocp.rearrange("p (cb n) -> p cb n", cb=2)[:, :, 0:400]
                    res = xa.rearrange(
                        "p (cb n) -> p cb n", cb=2)[:, :, w * 400:w * 400 + 400]
                    nc.vector.tensor_tensor(dst, src, res, op=ADD)
            for w in range(2):
                gcp = ps2g.tile([128, 128], F32, tag="gcp", name="gcp")
                for s in range(100):
                    nc.tensor.matmul(
                        gcp[:], lhsT=Xws[w][:, s * 128:(s + 1) * 128],
                        rhs=Xws[w][:, s * 128:(s + 1) * 128],
                        start=(s == 0), stop=(s == 99))
                if w == 0:
                    nc.vector.tensor_copy(gacc[:], gcp[:])
                else:
                    nc.vector.tensor_tensor(gacc[:], gacc[:], gcp[:], op=ADD)
            gcs = sb2n.tile([128, 128], F32, tag="gcs", name="gcs")
            nc.scalar.copy(gcs[:], gacc[:])
            nc.sync.dma_start(O["gcam"][t], gcs[:])


_PROG = None


def _get_prog():
    global _PROG
    if _PROG is None:
        nc = bacc.Bacc("TRN2", target_bir_lowering=False, debug=False,
                       num_devices=NCORES)
        I = {n: nc.dram_tensor(n, list(s[0]), s[1], kind="ExternalInput").ap()
             for n, s in IN_SPECS.items()}
        O = {n: nc.dram_tensor(n, list(s), F32, kind="ExternalOutput").ap()
             for n, s in OUT_SPECS.items()}
        _emit_program(nc, I, O)
        nc.compile()
        _PROG = nc
    return _PROG


# --------------------------------------------------------------------------
# host side
# --------------------------------------------------------------------------

def _make_in_maps(feat_S, feat_T, Wq, bq, Wk, bk, Wv, bv, gammacam, gammapam):
    gp = float(np.asarray(gammapam).reshape(-1)[0])
    gc = float(np.asarray(gammacam).reshape(-1)[0])
    gbv = (gp * np.asarray(bv, np.float32)).astype(np.float32)
    BF = ml_dtypes.bfloat16

    # chunk-major global rearrange: [25, 2, 16, 2, 128, 256] f32
    A = np.empty((25, 2, 16, 256, 256), np.float32)
    for t, X in enumerate((feat_S, feat_T)):
        A[:, t] = (np.asarray(X, np.float32)
                   .reshape(B, C, 5, 16, 5, 16)
                   .transpose(2, 4, 0, 1, 3, 5)
                   .reshape(25, B, C, 256))
    A = A.reshape(25, 2, 16, 2, 128, 256)

    consts = {
        "wqT": np.ascontiguousarray(
            np.asarray(Wq, np.float32).T.reshape(2, 128, CK)).astype(BF),
        "wkT": np.ascontiguousarray(
            np.asarray(Wk, np.float32).T.reshape(2, 128, CK)).astype(BF),
        "wvT": np.ascontiguousarray(
            (gp * np.asarray(Wv, np.float32)).T.reshape(2, 128, C)).astype(BF),
        "bq4": np.ascontiguousarray(np.tile(np.asarray(bq, np.float32), 4)[:, None]),
        "i128": np.eye(128, dtype=np.float32),
        "i128b": np.eye(128, dtype=np.float32).astype(BF),
        "i128r": np.eye(128, dtype=np.float32),
        "gicam": (gc * np.eye(128)).astype(np.float32),
        "gbv512": np.ascontiguousarray(np.tile(gbv, (128, 2))),
    }

    in_maps = []
    for j in range(NCORES):
        m = dict(consts)
        m["xc"] = A[3 * j:3 * j + 3]
        m["x24o"] = np.ascontiguousarray(A[24][:, 2 * j:2 * j + 2]).astype(BF)
        m["x24s"] = np.ascontiguousarray(A[24][:, :, :, :, 32 * j:32 * j + 32])
        in_maps.append(m)
    return in_maps


def _diag16(gfull):
    """gfull: [..., 128, 128] partials; f64-sum partials then diagonal blocks."""
    gf = gfull.astype(np.float64).reshape(-1, 128, 128).sum(axis=0)
    g = np.zeros((16, 16), np.float64)
    for r in range(8):
        g += gf[16 * r:16 * r + 16, 16 * r:16 * r + 16]
    return g


def _cka_loss(KS, KT):
    def cgram(K):
        rm = K.mean(axis=1, keepdims=True)
        cm = K.mean(axis=0, keepdims=True)
        return K - rm - cm + K.mean()
    cX, cY = cgram(KS), cgram(KT)
    hsic = float((cX * cY).sum())
    v1 = float(np.sqrt((cX * cX).sum()))
    v2 = float(np.sqrt((cY * cY).sum()))
    return -np.log(np.abs(hsic / (v1 * v2)) + 1e-8)


def _postprocess(results):
    losses = []
    for c in range(24):
        j, ci = divmod(c, 3)
        res = results[j]
        KS = _diag16(res["gpam"][ci, 0])
        KT = _diag16(res["gpam"][ci, 1])
        losses.append(_cka_loss(KS, KT))
    # chunk 24 on host
    FS = np.empty((B, 2 * 128 * 256), np.float32)
    FT = np.empty((B, 2 * 128 * 256), np.float32)
    for j in range(NCORES):
        for bo in range(2):
            FS[2 * j + bo] = results[j]["c24r"][0, bo].reshape(-1)
            FT[2 * j + bo] = results[j]["c24r"][1, bo].reshape(-1)
    FS = FS.astype(np.float64)
    FT = FT.astype(np.float64)
    KS24 = FS @ FS.T
    KT24 = FT @ FT.T
    losses.append(_cka_loss(KS24, KT24))
    loss_PAM = float(np.mean(losses))

    KSc = np.zeros((16, 16), np.float64)
    KTc = np.zeros((16, 16), np.float64)
    for j in range(NCORES):
        KSc += _diag16(results[j]["gcam"][0])
        KTc += _diag16(results[j]["gcam"][1])
    loss_CAM = float(_cka_loss(KSc, KTc))
    return np.float32(loss_CAM), np.float32(loss_PAM)


def _run_sim(nc, in_maps):
    from concourse.bass_interp import MultiCoreSim
    sim = MultiCoreSim(nc, num_cores=NCORES)
    cores = list(sim.cores.values())
    for j, core in enumerate(cores):
        for name, arr in in_maps[j].items():
            core.tensor(name)[:] = arr
    sim.simulate()
    return [{n: core.tensor(n).copy() for n in OUT_SPECS} for core in cores]


_LAST_EXEC_NS = None


def kernel(**inputs):
    global _LAST_EXEC_NS
    nc = _get_prog()
    in_maps = _make_in_maps(**{k: np.asarray(v) for k, v in inputs.items()})
    if os.environ.get("CRIT_BACKEND", "hw") == "sim":
        results = _run_sim(nc, in_maps)
    else:
        res = bass_utils.run_bass_kernel_spmd(
            nc, in_maps, core_ids=list(range(NCORES)),
            trace=os.environ.get("CRIT_TRACE", "0") == "1")
        results = res.results
        _LAST_EXEC_NS = res.exec_time_ns
    return _postprocess(results)
